# revision 1
# baseline (speedup 1.0000x reference)
"""Trainium2 Bass kernel for DeepSeek-style MLA (multi-head latent attention).

Sharding: 8 cores = 2 (batch) x 4 (head-groups of 4 heads).
Core c handles batch b = c // 4 and heads [4*(c%4), 4*(c%4)+4).
Each core computes its 4 heads' full attention + its partial o_proj
contribution y_partial [S, D] (bf16); host sums the 4 partials per batch.

v2 design (vs v1 baseline at ~582us):
  - All matmul operands bf16 (fp32 PSUM accumulation everywhere).
  - Merged projection phase A: one pass over hT per q-super computes the
    Q projection (+ shared k_pe) and the latent ckv^T DIRECTLY in
    transposed layout, eliminating v1's 64 PE transposes and ckv DRAM
    bounce.  kv_b (phase B) is emitted one super behind A inside the same
    loop so its PSUM tiles interleave and the PE never drains.
  - Partition broadcasts (RMS 1/rms, softmax 1/sum) via all-ones [P,P]
    lhsT matmuls: the column-sum matmul itself lands the broadcast rows
    in PSUM (no DRAM bounce, no extra bcast matmul).  RMS rsqrt comes
    straight from ACT Rsqrt (same act table as Copy).
  - Softmax denominators: exp tiles accumulate into an SBUF f32r tile
    (DVE); ONE ones-matmul per (h,qt) replaces v1's per-k-tile PE sum
    matmuls (-74K PE cycles).  1/sum via reciprocal_approx_fast (1 DVE op).
  - Exact 128-granular causal narrowing: diagonal-super score matmuls,
    exp, mask, E-accumulate and PV run on the valid [jl*128:512) columns
    only (-37K PE cycles); PSUM stop only on the last bank-touching mm.
  - Attention output normalization fused into the PSUM drain; o_proj
    interleaved one q-super behind attention; everything stays in SBUF.
  - y output in bf16 (halves the output DMA); host accumulates in fp32.
"""

import math
import sys

import numpy as np

for _p in ("/opt/trn_rl_repo",):
    if _p not in sys.path:
        sys.path.insert(0, _p)

# ---- problem constants (hardcoded per contract) ----
B = 2
S = 2048
D = 2048
H = 16
NOPE = 128
ROPE = 64
VD = 128
KV_RANK = 512
QHD = NOPE + ROPE
EPS = 1e-6
BASE = 10000.0

HPC = 4            # heads per core
NCORES = 8
P = 128
QS = 512           # q-super width
NQ = S // QS       # 4
NST = S // P       # 16 s-tiles
NKC = D // P       # 16 d-chunks
RC = KV_RANK // P  # 4 r-chunks
HALF = ROPE // 2   # 32

# wA column layout: 4x q-nope(128) | 2x q-pe pair(128) | kpe-w(64) | ckv-w(512)
WQ = HPC * QHD       # 768 (q cols)
WKPE = WQ            # offset of kpe cols
WCKV = WQ + ROPE     # 832, offset of ckv cols
WCOLS = WCKV + KV_RANK  # 1344
# contiguous weight DMA blocks (col offset, width), in consumption order
WBLOCKS = [(0, 256), (256, 256), (512, 256), (768, 192), (960, 256), (1216, 128)]

_CACHE = {}


def _declare_io(nc):
    import concourse.mybir as mybir

    f32 = mybir.dt.float32
    bf16 = mybir.dt.bfloat16
    io = {}
    io["hT"] = nc.dram_tensor("hT", [D, S], bf16, kind="ExternalInput").ap()
    io["wA"] = nc.dram_tensor("wA", [D * WCOLS], bf16, kind="ExternalInput").ap()
    io["kvbk"] = nc.dram_tensor("kvbk", [P, RC, HPC * NOPE], bf16, kind="ExternalInput").ap()
    io["kvbv"] = nc.dram_tensor("kvbv", [P, RC, HPC * VD], bf16, kind="ExternalInput").ap()
    io["owT"] = nc.dram_tensor("owT", [P, HPC, D], bf16, kind="ExternalInput").ap()
    io["cosT2"] = nc.dram_tensor("cosT2", [P, S], f32, kind="ExternalInput").ap()
    io["sinT2"] = nc.dram_tensor("sinT2", [P, S], f32, kind="ExternalInput").ap()
    io["masks"] = nc.dram_tensor("masks", [P, 4, QS], bf16, kind="ExternalInput").ap()
    io["y"] = nc.dram_tensor("y", [S, D], bf16, kind="ExternalOutput").ap()
    return io


def _emit(nc, tc, io, sfx=""):
    """Emit the whole per-core program into TileContext tc."""
    import concourse.mybir as mybir

    f32 = mybir.dt.float32
    fr = mybir.dt.float32r
    bf16 = mybir.dt.bfloat16
    AF = mybir.ActivationFunctionType

    hT = io["hT"]; wA = io["wA"]
    kvbk = io["kvbk"]; kvbv = io["kvbv"]; owT = io["owT"]
    cosT2 = io["cosT2"]; sinT2 = io["sinT2"]; masks = io["masks"]
    y = io["y"]

    # ---- long-lived pools, strictly nested (LIFO exit) ----
    p_const_cm = tc.tile_pool(name="const" + sfx, bufs=1)
    p_const = p_const_cm.__enter__()
    p_qt_cm = tc.tile_pool(name="qt" + sfx, bufs=1)        # QTn/QTp/kpeT
    p_qt = p_qt_cm.__enter__()
    p_kvb_cm = tc.tile_pool(name="kvb" + sfx, bufs=1)      # KTn/Vsb: B -> attn
    p_kvb = p_kvb_cm.__enter__()
    p_bw_cm = tc.tile_pool(name="Bw" + sfx, bufs=1)        # kv_b weights
    p_bw = p_bw_cm.__enter__()
    p_ckv_cm = tc.tile_pool(name="ckv" + sfx, bufs=1)      # ckvT: A -> B
    p_ckv = p_ckv_cm.__enter__()
    p_cs_cm = tc.tile_pool(name="cossin" + sfx, bufs=1)    # cos/sin: A only
    p_cs = p_cs_cm.__enter__()

    ones_pp_f = p_const.tile([P, P], f32, name="ones_pp")
    nc.vector.memset(ones_pp_f, 1.0)
    ones_pp = ones_pp_f.bitcast(fr)
    eps_sb = p_const.tile([P, 1], f32, name="eps")
    nc.vector.memset(eps_sb, EPS)
    mask_sb = p_const.tile([P, 4, QS], bf16, name="masks")

    QTn = p_qt.tile([P, HPC, S], bf16, name="QTn")
    QTp = p_qt.tile([P, 2, S], bf16, name="QTp")
    # kpeT duplicated on partitions [0:64] and [64:128] (lhsT base must
    # match rhs base per head parity)
    kpeT = p_qt.tile([P, S], bf16, name="kpeT")
    ckvT = p_ckv.tile([P, RC, S], bf16, name="ckvT")
    KTn = p_kvb.tile([P, HPC, S], bf16, name="KTn")
    Vsb = p_kvb.tile([P, NST, HPC, VD], bf16, name="Vsb")
    kvbk_sb = p_bw.tile([P, RC, HPC * NOPE], bf16, name="kvbk")
    kvbv_sb = p_bw.tile([P, RC, HPC * VD], bf16, name="kvbv")
    cos_sb = p_cs.tile([P, S], f32, name="cos")
    sin_sb = p_cs.tile([P, S], f32, name="sin")

    # ============ Phase A+B: projections, one super at a time =============
    # A groups (pairs of m-tiles, 2 banks each): [qn0,qn1] [qn2,qn3]
    # [pe0,pe1] [kpe,ckv0] [ckv1,ckv2] [ckv3,-].  PSUM: psA 2x2 + stats 1
    # + psB 3 = 8 banks.  kv_b work for super sc is emitted after super
    # sc+1's A groups so the PE never waits on the normalize chain.
    with tc.tile_pool(name="Ah" + sfx, bufs=3) as p_hq, \
         tc.tile_pool(name="Aw" + sfx, bufs=1) as p_wa, \
         tc.tile_pool(name="Ar" + sfx, bufs=2) as p_rope, \
         tc.tile_pool(name="Asq" + sfx, bufs=2) as p_sq, \
         tc.tile_pool(name="Ars" + sfx, bufs=2) as p_rs, \
         tc.tile_pool(name="psA" + sfx, bufs=2, space="PSUM") as psA, \
         tc.tile_pool(name="psS" + sfx, bufs=1, space="PSUM") as psS, \
         tc.tile_pool(name="psB" + sfx, bufs=3, space="PSUM") as psB:

        wa_sb = p_wa.tile([P, NKC, WCOLS], bf16, name="wa")

        def wa_dma(b, half=None):
            c0, cw = WBLOCKS[b]
            off = D * c0
            src_ = wA[off:off + D * cw].rearrange(
                "(kk p c) -> p kk c", p=P, c=cw)
            if half is None:
                nc.sync.dma_start(out=wa_sb[:, :, c0:c0 + cw], in_=src_)
            else:
                k0, k1 = (0, NKC // 2) if half == 0 else (NKC // 2, NKC)
                nc.sync.dma_start(out=wa_sb[:, k0:k1, c0:c0 + cw],
                                  in_=src_[:, k0:k1, :])

        def hq_dma(sc, half, split=False):
            t = p_hq.tile([P, NKC // 2, QS], bf16, name="hq")
            src = hT[half * 1024:(half + 1) * 1024,
                     sc * QS:(sc + 1) * QS].rearrange("(kk p) s -> p kk s", p=P)
            if split:  # startup: land the first k-chunks earlier
                nc.sync.dma_start(out=t[:, 0:2, :], in_=src[:, 0:2, :])
                nc.sync.dma_start(out=t[:, 2:4, :], in_=src[:, 2:4, :])
                nc.sync.dma_start(out=t[:, 4:8, :], in_=src[:, 4:8, :])
            else:
                nc.sync.dma_start(out=t, in_=src)
            return t

        # startup interleave in consumption order so the PE never starves
        wa_dma(0, half=0)
        pend = [[hq_dma(0, 0, split=True)]]
        wa_dma(0, half=1)
        pend[0].append(hq_dma(0, 1))
        wa_dma(1)
        wa_dma(2)
        pend.append([hq_dma(1, 0)])
        wa_dma(3)
        wa_dma(4)
        pend[1].append(hq_dma(1, 1))
        wa_dma(5)
        nc.sync.dma_start(out=cos_sb, in_=cosT2)
        nc.sync.dma_start(out=sin_sb, in_=sinT2)
        pend.append([hq_dma(2, 0), hq_dma(2, 1)])
        # prefetch phase-B weights + masks during A
        for rc in range(RC):
            nc.sync.dma_start(out=kvbk_sb[:, rc, :], in_=kvbk[:, rc, :])
            nc.sync.dma_start(out=kvbv_sb[:, rc, :], in_=kvbv[:, rc, :])
        nc.sync.dma_start(out=mask_sb, in_=masks)

        # m-tile groups: (wA col offset, width, kind, idx)
        GROUPS = [
            [("qn", 0), ("qn", 1)],
            [("qn", 2), ("qn", 3)],
            [("pe", 0), ("pe", 1)],
            [("kpe", 0), ("ckv", 0)],
            [("ckv", 1), ("ckv", 2)],
            [("ckv", 3)],
        ]

        def mcol(kind, idx):
            if kind == "qn":
                return idx * P, P
            if kind == "pe":
                return (4 + idx) * P, P
            if kind == "kpe":
                return WKPE, ROPE
            return WCKV + idx * P, P

        def rope_psum(src, part_hi, q0, dsts):
            """RoPE from PSUM src [part_hi, QS] -> each dst slice (bf16).
            cos/sin rows repeat every 64 partitions."""
            rot = p_rope.tile([P, QS], f32, name="rot")
            t1 = p_rope.tile([P, QS], f32, name="t1")
            for b0 in range(0, part_hi, ROPE):
                nc.vector.tensor_scalar_mul(
                    out=rot[b0:b0 + HALF], in0=src[b0 + HALF:b0 + ROPE],
                    scalar1=-1.0)
                nc.vector.tensor_copy(
                    out=rot[b0 + HALF:b0 + ROPE], in_=src[b0:b0 + HALF])
            csl = cos_sb[:part_hi, q0:q0 + QS]
            ssl = sin_sb[:part_hi, q0:q0 + QS]
            nc.vector.tensor_mul(t1[:part_hi], src, csl)
            nc.vector.tensor_mul(rot[:part_hi], rot[:part_hi], ssl)
            for dst in dsts:
                nc.vector.tensor_add(dst, t1[:part_hi], rot[:part_hi])

        def emit_A_super(sc, hq):
            q0 = sc * QS
            ps_ss = psS.tile([P, QS], f32, name="ps_ss")
            for group in GROUPS:
                ps = psA.tile([P, 2, QS], f32, name="psA")
                for k in range(NKC):
                    rhs = hq[k // 8][:, k % 8, :]
                    for i, (kind, idx) in enumerate(group):
                        c0, cw = mcol(kind, idx)
                        nc.tensor.matmul(
                            ps[:cw, i, :], wa_sb[:, k, c0:c0 + cw], rhs,
                            start=(k == 0), stop=(k == NKC - 1))
                for i, (kind, idx) in enumerate(group):
                    if kind == "qn":
                        nc.scalar.copy(out=QTn[:, idx, q0:q0 + QS],
                                       in_=ps[:, i, :])
                    elif kind == "pe":
                        rope_psum(ps[:, i, :], P, q0,
                                  [QTp[:, idx, q0:q0 + QS]])
                    elif kind == "kpe":
                        rope_psum(ps[:ROPE, i, :], ROPE, q0,
                                  [kpeT[:ROPE, q0:q0 + QS],
                                   kpeT[ROPE:, q0:q0 + QS]])
                    else:  # ckv
                        rc = idx
                        sq = p_sq.tile([P, QS], fr, name="sq")
                        nc.scalar.activation(out=sq, in_=ps[:, i, :],
                                             func=AF.Square)
                        nc.tensor.matmul(ps_ss, ones_pp, sq,
                                         start=(rc == 0), stop=(rc == RC - 1))
                        nc.scalar.copy(out=ckvT[:, rc, q0:q0 + QS],
                                       in_=ps[:, i, :])
            # rs = 1/sqrt(mean + eps), already broadcast across partitions
            s_b = p_rs.tile([P, QS], f32, name="s_b")
            nc.scalar.activation(out=s_b, in_=ps_ss, func=AF.Sqrt,
                                 bias=eps_sb, scale=1.0 / KV_RANK)
            rs_b = p_rs.tile([P, QS], f32, name="rs_b")
            nc.vector.reciprocal_approx_fast(out=rs_b, in_=s_b)
            for rc in range(RC):
                nc.vector.tensor_mul(ckvT[:, rc, q0:q0 + QS],
                                     ckvT[:, rc, q0:q0 + QS], rs_b)

        def emit_B_super(sc):
            for h in range(HPC):
                ps = psB.tile([P, QS], f32, name="psb")
                for rc in range(RC):
                    nc.tensor.matmul(
                        ps, kvbk_sb[:, rc, h * NOPE:(h + 1) * NOPE],
                        ckvT[:, rc, sc * QS:(sc + 1) * QS],
                        start=(rc == 0), stop=(rc == RC - 1))
                nc.scalar.copy(out=KTn[:, h, sc * QS:(sc + 1) * QS], in_=ps)
            for stl in range(4):
                st = sc * 4 + stl
                psv = psB.tile([P, QS], f32, name="psb")
                for rc in range(RC):
                    nc.tensor.matmul(
                        psv, ckvT[:, rc, st * P:(st + 1) * P],
                        kvbv_sb[:, rc, :],
                        start=(rc == 0), stop=(rc == RC - 1))
                nc.scalar.copy(out=Vsb[:, st, :, :],
                               in_=psv.rearrange("p (h v) -> p h v", h=HPC))

        for sc in range(NQ):
            hq = pend.pop(0)
            if sc + 3 < NQ:
                pend.append([hq_dma(sc + 3, 0), hq_dma(sc + 3, 1)])
            emit_A_super(sc, hq)
            if sc >= 1:
                emit_B_super(sc - 1)
        emit_B_super(NQ - 1)

    p_cs_cm.__exit__(None, None, None)  # free cos/sin
    p_ckv_cm.__exit__(None, None, None)  # free ckvT

    p_ow_cm = tc.tile_pool(name="ow" + sfx, bufs=1)
    p_ow = p_ow_cm.__enter__()
    owT_sb = p_ow.tile([P, HPC, D], bf16, name="owT")
    p_at_cm = tc.tile_pool(name="at" + sfx, bufs=1)
    p_at = p_at_cm.__enter__()
    at = p_at.tile([P, HPC, S], bf16, name="at")
    for hc in range(HPC):
        nc.sync.dma_start(out=owT_sb[:, hc, :], in_=owT[:, hc, :])

    # ============ Phase C: attention + o_proj, per q-super ================
    # PSUM: psSC 3 + psPV 2 + psM 1 + psD 2 = 8 banks.
    with tc.tile_pool(name="Ce" + sfx, bufs=4) as pEP, \
         tc.tile_pool(name="Ca" + sfx, bufs=2) as pEacc, \
         tc.tile_pool(name="Cr" + sfx, bufs=2) as pRec, \
         tc.tile_pool(name="Dy" + sfx, bufs=4) as pDy, \
         tc.tile_pool(name="psC" + sfx, bufs=2, space="PSUM") as psSC, \
         tc.tile_pool(name="psP" + sfx, bufs=3, space="PSUM") as psPV, \
         tc.tile_pool(name="psM" + sfx, bufs=1, space="PSUM") as psM, \
         tc.tile_pool(name="psD" + sfx, bufs=2, space="PSUM") as psD:

        def emit_oproj_st(qt, stl):
            st = qt * 4 + stl
            for nk in range(NQ):
                psy = psD.tile([P, QS], f32, name="psy")
                for hc in range(HPC):
                    nc.tensor.matmul(
                        psy, at[:, hc, st * P:(st + 1) * P],
                        owT_sb[:, hc, nk * QS:(nk + 1) * QS],
                        start=(hc == 0), stop=(hc == HPC - 1))
                ys = pDy.tile([P, QS], bf16, name="ys")
                nc.scalar.copy(out=ys, in_=psy)
                nc.gpsimd.dma_start(
                    out=y[st * P:(st + 1) * P, nk * QS:(nk + 1) * QS],
                    in_=ys)

        SEQ = [1, 2, 3, 0]
        for qi, qt in enumerate(SEQ):
            q0 = qt * QS
            nj = 4 * qt + 4
            for h in range(HPC):
                hp = (h % 2) * ROPE
                qprhs = QTp[hp:hp + ROPE, h // 2, :]
                E_acc = pEacc.tile([P, QS], fr, name="E_acc")
                ps_pv = psPV.tile([P, QS], f32, name="ps_pv")
                for j in range(nj):
                    jl = j - 4 * qt
                    off = max(jl, 0) * P
                    ps_sc = psSC.tile([P, QS], f32, name="ps_sc")
                    nc.tensor.matmul(
                        ps_sc[:, off:], KTn[:, h, j * P:(j + 1) * P],
                        QTn[:, h, q0 + off:q0 + QS], start=True, stop=False)
                    nc.tensor.matmul(
                        ps_sc[:, off:],
                        kpeT[hp:hp + ROPE, j * P:(j + 1) * P],
                        qprhs[:, q0 + off:q0 + QS], start=False, stop=True)
                    ep = pEP.tile([P, QS], bf16, name="ep")
                    nc.scalar.activation(out=ep[:, off:], in_=ps_sc[:, off:],
                                         func=AF.Exp)
                    if jl >= 0:  # diagonal super-block: causal mask
                        nc.vector.tensor_mul(ep[:, off:], ep[:, off:],
                                             mask_sb[:, jl, off:])
                    if j == 0:
                        nc.vector.tensor_copy(out=E_acc, in_=ep)
                    else:
                        nc.vector.tensor_add(E_acc[:, off:], E_acc[:, off:],
                                             ep[:, off:])
                    nc.tensor.matmul(ps_pv[:, off:], Vsb[:, j, h, :],
                                     ep[:, off:], start=(j == 0),
                                     stop=(jl == 3))
                # broadcast column sums on PE, 1/x on DVE, fused drain
                ps_sums = psM.tile([P, QS], f32, name="ps_sums")
                nc.tensor.matmul(ps_sums, ones_pp, E_acc, start=True, stop=True)
                rec = pRec.tile([P, QS], f32, name="rec")
                nc.vector.reciprocal_approx_fast(out=rec, in_=ps_sums)
                nc.vector.tensor_mul(at[:, h, q0:q0 + QS], ps_pv, rec)
                if qi >= 1:
                    emit_oproj_st(SEQ[qi - 1], h)
        for stl in range(4):
            emit_oproj_st(SEQ[-1], stl)

    p_at_cm.__exit__(None, None, None)
    p_ow_cm.__exit__(None, None, None)
    p_bw_cm.__exit__(None, None, None)
    p_kvb_cm.__exit__(None, None, None)
    p_qt_cm.__exit__(None, None, None)
    p_const_cm.__exit__(None, None, None)


def _build_program(reps=1):
    import concourse.bacc as bacc
    import concourse.tile as tile

    nc = bacc.Bacc("TRN2", target_bir_lowering=False, debug=False,
                   num_devices=NCORES)
    with tile.TileContext(nc) as tc:
        io = _declare_io(nc)
        for r in range(reps):
            _emit(nc, tc, io, sfx=f"_r{r}" if reps > 1 else "")
    nc.compile()
    return nc


def _rope_cos_sin():
    inv_freq = 1.0 / (BASE ** (np.arange(0, ROPE, 2, dtype=np.float32) / ROPE))
    t = np.arange(S, dtype=np.float32)
    freqs = np.outer(t, inv_freq)                     # [S, ROPE/2]
    emb = np.concatenate([freqs, freqs], axis=-1)     # [S, ROPE]
    return np.cos(emb), np.sin(emb)


def _host_prep(hidden_states, q_proj_w, kv_a_proj_w, kv_a_norm_w,
               kv_b_proj_w, o_proj_w):
    """Build per-core input maps (bf16 operands)."""
    import ml_dtypes

    bf = ml_dtypes.bfloat16
    hidden_states = np.asarray(hidden_states, dtype=np.float32)
    q_proj_w = np.asarray(q_proj_w, dtype=np.float32)
    kv_a_proj_w = np.asarray(kv_a_proj_w, dtype=np.float32)
    kv_a_norm_w = np.asarray(kv_a_norm_w, dtype=np.float32)
    kv_b_proj_w = np.asarray(kv_b_proj_w, dtype=np.float32)
    o_proj_w = np.asarray(o_proj_w, dtype=np.float32)

    scale = np.float32(1.0 / math.sqrt(QHD))
    qws = (q_proj_w * scale).reshape(H, QHD, D)
    kvb = (kv_b_proj_w * kv_a_norm_w[None, :]).reshape(H, NOPE + VD, KV_RANK)

    cos, sin = _rope_cos_sin()                             # [S, ROPE]
    cosT2 = np.ascontiguousarray(np.tile(cos.T, (2, 1)))   # [128, S] f32
    sinT2 = np.ascontiguousarray(np.tile(sin.T, (2, 1)))

    # diag masks, stored partition-major: masks[p, j, q]
    r = np.arange(P)[:, None]
    ql = np.arange(QS)[None, :]
    masks = np.stack([(ql >= j * P + r).astype(np.float32) for j in range(4)])
    masks = np.ascontiguousarray(masks.transpose(1, 0, 2)).astype(bf)

    in_maps = []
    for c in range(NCORES):
        b, g = divmod(c, HPC)
        heads = list(range(HPC * g, HPC * g + HPC))
        hT = np.ascontiguousarray(hidden_states[b].T).astype(bf)   # [D, S]
        # wA cols: 4x nope(128), 2x pe-pair(128), kpe-w(64), ckv-w(512)
        cols = np.concatenate(
            [qws[h, :NOPE, :] for h in heads]
            + [qws[h, NOPE:, :] for h in heads]
            + [kv_a_proj_w[KV_RANK:, :], kv_a_proj_w[:KV_RANK, :]],
            axis=0)                                        # [1344, D]
        wA_full = cols.T                                   # [D, 1344]
        # flat, block-contiguous in WBLOCKS order (device DMAs per block)
        wA_c = np.concatenate(
            [np.ascontiguousarray(wA_full[:, c0:c0 + cw]).ravel()
             for c0, cw in WBLOCKS]).astype(bf)
        # kvbk [128, 4, 4*128]: kvbk[p, rc, h*128+j] = kvb[heads[h], j, rc*128+p]
        kn = np.stack([kvb[h, :NOPE, :] for h in heads])    # [h, j, r]
        kvbk_c = np.ascontiguousarray(
            kn.transpose(2, 0, 1).reshape(RC, P, HPC, NOPE)
            .transpose(1, 0, 2, 3).reshape(P, RC, HPC * NOPE)).astype(bf)
        kv = np.stack([kvb[h, NOPE:, :] for h in heads])    # [h, j(vd), r]
        kvbv_c = np.ascontiguousarray(
            kv.transpose(2, 0, 1).reshape(RC, P, HPC, VD)
            .transpose(1, 0, 2, 3).reshape(P, RC, HPC * VD)).astype(bf)
        # owT [128, 4, D]: owT[p, hc, n] = o_proj_w[n, g*512 + hc*128 + p]
        ow = o_proj_w[:, g * HPC * VD:(g + 1) * HPC * VD]   # [D, 512]
        owT_c = np.ascontiguousarray(
            ow.T.reshape(HPC, VD, D).transpose(1, 0, 2)).astype(bf)
        in_maps.append({
            "hT": hT, "wA": wA_c,
            "kvbk": kvbk_c, "kvbv": kvbv_c, "owT": owT_c,
            "cosT2": cosT2, "sinT2": sinT2, "masks": masks,
        })
    return in_maps


def _gather(results):
    out = np.zeros((B, S, D), dtype=np.float32)
    for c in range(NCORES):
        out[c // HPC] += np.asarray(results[c]["y"], dtype=np.float32)
    return out


def kernel(hidden_states, q_proj_w, kv_a_proj_w, kv_a_norm_w,
           kv_b_proj_w, o_proj_w):
    from concourse import bass_utils

    in_maps = _host_prep(hidden_states, q_proj_w, kv_a_proj_w, kv_a_norm_w,
                         kv_b_proj_w, o_proj_w)
    if "nc" not in _CACHE:
        _CACHE["nc"] = _build_program()
    nc = _CACHE["nc"]
    res = bass_utils.run_bass_kernel_spmd(nc, in_maps, list(range(NCORES)))
    return _gather(res.results)


if __name__ == "__main__":
    rng = np.random.default_rng(0)
    ins = {
        "hidden_states": rng.standard_normal((B, S, D), dtype=np.float32),
        "q_proj_w": rng.standard_normal((H * QHD, D), dtype=np.float32) * D ** -0.5,
        "kv_a_proj_w": rng.standard_normal((KV_RANK + ROPE, D), dtype=np.float32) * D ** -0.5,
        "kv_a_norm_w": np.ones(KV_RANK, dtype=np.float32),
        "kv_b_proj_w": rng.standard_normal((H * (NOPE + VD), KV_RANK), dtype=np.float32) * KV_RANK ** -0.5,
        "o_proj_w": rng.standard_normal((D, H * VD), dtype=np.float32) * (H * VD) ** -0.5,
    }
    out = kernel(**ins)
    print(out.shape, out.dtype, float(np.abs(out).mean()))



# revision 18
# speedup vs baseline: 59.6149x; 59.6149x over previous
"""Trainium2 Bass kernel for DeepSeek-style MLA (multi-head latent attention).

Sharding: 8 cores = 2 (batch) x 4 (head-groups of 4 heads).
Core c handles batch b = c // 4 and heads [4*(c%4), 4*(c%4)+4).
Each core computes its 4 heads' full attention + its partial o_proj
contribution y_partial [S, D] (bf16); host sums the 4 partials per batch.

v3 design (vs v2 baseline at ~295us):
  - Latent (ckv) projection sharded 4-way across the cores of each batch
    group: core c computes only ckv rows [128*(c%4), 128*(c%4)+128) (the
    chunk is baked into its wA input data; the program stays SPMD-pure).
    Per q-super, chunks are exchanged with an HBM AllGather over replica
    groups [[0..3],[4..7]] and read back in global chunk order, so no
    instruction depends on the core id.  This removes 4.6 GFLOP/core of
    redundant projection work (A phase: 11 m-tiles -> 8 per super).
  - RMS norm moved after the gather (sum-of-squares via ones-matmul over
    the 4 gathered chunks, rsqrt on ACT, scale on DVE) inside phase B.
  - Phase C starts with q-super 0 (needs only B(0)) and B(3) is emitted
    inside phase C behind C(qt=0), hiding the last gather's latency.
  - Everything else follows v2: bf16 matmuls with fp32 PSUM, partition
    broadcasts via all-ones lhsT matmuls, exact 128-granular causal
    narrowing, o_proj interleaved one q-super behind attention.
"""

import math
import sys

import numpy as np

for _p in ("/opt/trn_rl_repo",):
    if _p not in sys.path:
        sys.path.insert(0, _p)

# ---- problem constants (hardcoded per contract) ----
B = 2
S = 2048
D = 2048
H = 16
NOPE = 128
ROPE = 64
VD = 128
KV_RANK = 512
QHD = NOPE + ROPE
EPS = 1e-6
BASE = 10000.0

HPC = 4            # heads per core
NCORES = 8
P = 128
QS = 512           # q-super width
NQ = S // QS       # 4
NST = S // P       # 16 s-tiles
NKC = D // P       # 16 d-chunks
RC = KV_RANK // P  # 4 r-chunks
HALF = ROPE // 2   # 32

# wA column layout: 4x q-nope(128) | 2x q-pe pair(128) | kpe-w(64) | ckv-chunk(128)
WQ = HPC * QHD       # 768 (q cols)
WKPE = WQ            # 768, offset of kpe cols
WCKV = WQ + ROPE     # 832, offset of this core's ckv chunk cols
WCOLS = WCKV + P     # 960
# contiguous weight DMA blocks (col offset, width), in consumption order
WBLOCKS = [(0, 256), (256, 256), (512, 256), (768, 192)]

CC_GROUPS = [[0, 1, 2, 3], [4, 5, 6, 7]]

_CACHE = {}


def _declare_io(nc):
    import concourse.mybir as mybir

    f32 = mybir.dt.float32
    bf16 = mybir.dt.bfloat16
    io = {}
    io["hT"] = nc.dram_tensor("hT", [D, S], bf16, kind="ExternalInput").ap()
    io["wA"] = nc.dram_tensor("wA", [D * WCOLS], bf16, kind="ExternalInput").ap()
    io["kvbk"] = nc.dram_tensor("kvbk", [P, RC, HPC * NOPE], bf16, kind="ExternalInput").ap()
    io["kvbv"] = nc.dram_tensor("kvbv", [P, RC, HPC * VD], bf16, kind="ExternalInput").ap()
    io["owT"] = nc.dram_tensor("owT", [P, HPC, D], bf16, kind="ExternalInput").ap()
    io["cosT2"] = nc.dram_tensor("cosT2", [P, S], f32, kind="ExternalInput").ap()
    io["sinT2"] = nc.dram_tensor("sinT2", [P, S], f32, kind="ExternalInput").ap()
    io["masks"] = nc.dram_tensor("masks", [P, 4, QS], bf16, kind="ExternalInput").ap()
    io["y"] = nc.dram_tensor("y", [S, D], bf16, kind="ExternalOutput").ap()
    return io


def _declare_cc(nc, sfx=""):
    import concourse.mybir as mybir

    bf16 = mybir.dt.bfloat16
    part = nc.dram_tensor("ckv_part" + sfx, [NQ, P, QS], bf16, kind="Internal").ap()
    gath = nc.dram_tensor("ckv_gath" + sfx, [NQ, RC, P, QS], bf16, kind="Internal").ap()
    return part, gath


def _emit(nc, tc, io, sfx=""):
    """Emit the whole per-core program into TileContext tc."""
    import concourse.mybir as mybir

    f32 = mybir.dt.float32
    fr = mybir.dt.float32r
    bf16 = mybir.dt.bfloat16
    AF = mybir.ActivationFunctionType

    hT = io["hT"]; wA = io["wA"]
    kvbk = io["kvbk"]; kvbv = io["kvbv"]; owT = io["owT"]
    cosT2 = io["cosT2"]; sinT2 = io["sinT2"]; masks = io["masks"]
    y = io["y"]
    ccp, ccg = _declare_cc(nc, sfx)

    # ---- long-lived pools, strictly nested (LIFO exit) ----
    p_const_cm = tc.tile_pool(name="const" + sfx, bufs=1)
    p_const = p_const_cm.__enter__()
    p_qt_cm = tc.tile_pool(name="qt" + sfx, bufs=1)        # QTn/QTp/kpeT
    p_qt = p_qt_cm.__enter__()
    p_kvb_cm = tc.tile_pool(name="kvb" + sfx, bufs=1)      # KTn/Vsb: B -> attn
    p_kvb = p_kvb_cm.__enter__()
    p_bw_cm = tc.tile_pool(name="Bw" + sfx, bufs=1)        # kv_b weights
    p_bw = p_bw_cm.__enter__()
    p_ckv_cm = tc.tile_pool(name="ckv" + sfx, bufs=1)      # ckvT: gather -> B
    p_ckv = p_ckv_cm.__enter__()
    p_cs_cm = tc.tile_pool(name="cossin" + sfx, bufs=1)    # cos/sin: A only
    p_cs = p_cs_cm.__enter__()

    ones_pp_f = p_const.tile([P, P], f32, name="ones_pp")
    nc.vector.memset(ones_pp_f, 1.0)
    ones_pp = ones_pp_f.bitcast(fr)
    eps_sb = p_const.tile([P, 1], f32, name="eps")
    nc.vector.memset(eps_sb, EPS)
    mask_sb = p_const.tile([P, 4, QS], bf16, name="masks")

    QTn = p_qt.tile([P, HPC, S], bf16, name="QTn")
    QTp = p_qt.tile([P, 2, S], bf16, name="QTp")
    # kpeT duplicated on partitions [0:64] and [64:128] (lhsT base must
    # match rhs base per head parity)
    kpeT = p_qt.tile([P, S], bf16, name="kpeT")
    ckvT = p_ckv.tile([P, RC, S], bf16, name="ckvT")
    KTn = p_kvb.tile([P, HPC, S], bf16, name="KTn")
    Vsb = p_kvb.tile([P, NST, HPC, VD], bf16, name="Vsb")
    kvbk_sb = p_bw.tile([P, RC, HPC * NOPE], bf16, name="kvbk")
    kvbv_sb = p_bw.tile([P, RC, HPC * VD], bf16, name="kvbv")
    cos_sb = p_cs.tile([P, S], f32, name="cos")
    sin_sb = p_cs.tile([P, S], f32, name="sin")

    # ============ Phase A: projections, one super at a time =============
    # A groups (pairs of m-tiles, 2 banks each): [qn0,qn1] [qn2,qn3]
    # [pe0,pe1] [kpe,ckv_own].  PSUM: psA 2x2 + stats 1 + psB 3 = 8 banks.
    # After each super's A groups, the core's ckv chunk is pushed to HBM
    # and AllGather'ed; phase B for super sc is emitted after super sc+1's
    # A groups so the gather latency hides behind projection work.  B(3)
    # is emitted later, behind C(qt=0), with its own nested pools.
    p_A_cms = [tc.tile_pool(name="Ah" + sfx, bufs=3),
               tc.tile_pool(name="Aw" + sfx, bufs=1),
               tc.tile_pool(name="Ar" + sfx, bufs=2),
               tc.tile_pool(name="Ack" + sfx, bufs=2),
               tc.tile_pool(name="Bsq" + sfx, bufs=2),
               tc.tile_pool(name="Brs" + sfx, bufs=2),
               tc.tile_pool(name="psA" + sfx, bufs=2, space="PSUM"),
               tc.tile_pool(name="psS" + sfx, bufs=1, space="PSUM"),
               tc.tile_pool(name="psB" + sfx, bufs=3, space="PSUM")]
    p_hq, p_wa, p_rope, p_ckvo, p_sq, p_rs, psA, psS, psB = [
        cm.__enter__() for cm in p_A_cms]

    wa_sb = p_wa.tile([P, NKC, WCOLS], bf16, name="wa")

    def wa_dma(b, half=None, eng=None):
        eng = eng or nc.sync
        c0, cw = WBLOCKS[b]
        off = D * c0
        src_ = wA[off:off + D * cw].rearrange(
            "(kk p c) -> p kk c", p=P, c=cw)
        if half is None:
            eng.dma_start(out=wa_sb[:, :, c0:c0 + cw], in_=src_)
        else:
            k0, k1 = (0, NKC // 2) if half == 0 else (NKC // 2, NKC)
            eng.dma_start(out=wa_sb[:, k0:k1, c0:c0 + cw],
                          in_=src_[:, k0:k1, :])

    def hq_dma(sc, half, split=False, eng=None):
        eng = eng or nc.sync
        t = p_hq.tile([P, NKC // 2, QS], bf16, name="hq")
        src = hT[half * 1024:(half + 1) * 1024,
                 sc * QS:(sc + 1) * QS].rearrange("(kk p) s -> p kk s", p=P)
        if split:  # startup: land the first k-chunks earlier, and spread
            # descriptor generation across two engine queues
            nc.scalar.dma_start(out=t[:, 0:4, :], in_=src[:, 0:4, :])
            nc.sync.dma_start(out=t[:, 4:8, :], in_=src[:, 4:8, :])
        else:
            eng.dma_start(out=t, in_=src)
        return t

    # startup: spread descriptor generation across idle engine queues so
    # super 0's operands ([kpe,ckv] group first: wa block 3, hq, cos/sin)
    # land in parallel rather than serializing ~3us each on SP.
    wa_dma(3, eng=nc.gpsimd)
    pend = [[hq_dma(0, 0, split=True)]]
    pend[0].append(hq_dma(0, 1, eng=nc.gpsimd))
    nc.gpsimd.dma_start(out=cos_sb, in_=cosT2)
    nc.gpsimd.dma_start(out=sin_sb, in_=sinT2)
    wa_dma(0, half=0)
    wa_dma(0, half=1)
    wa_dma(1)
    pend.append([hq_dma(1, 0)])
    wa_dma(2)
    pend[1].append(hq_dma(1, 1))
    pend.append([hq_dma(2, 0), hq_dma(2, 1)])
    # prefetch phase-B weights + masks during A
    for rc in range(RC):
        nc.sync.dma_start(out=kvbk_sb[:, rc, :], in_=kvbk[:, rc, :])
        nc.sync.dma_start(out=kvbv_sb[:, rc, :], in_=kvbv[:, rc, :])
    nc.sync.dma_start(out=mask_sb, in_=masks)

    # m-tile groups: (kind, idx); [kpe, ckv] first so the gather starts early
    GROUPS = [
        [("kpe", 0), ("ckv", 0)],
        [("qn", 0), ("qn", 1)],
        [("qn", 2), ("qn", 3)],
        [("pe", 0), ("pe", 1)],
    ]

    def mcol(kind, idx):
        if kind == "qn":
            return idx * P, P
        if kind == "pe":
            return (4 + idx) * P, P
        if kind == "kpe":
            return WKPE, ROPE
        return WCKV, P

    def rope_psum(src, part_hi, q0, dsts):
        """RoPE from PSUM src [part_hi, QS] -> each dst slice (bf16).
        cos/sin rows repeat every 64 partitions."""
        rot = p_rope.tile([P, QS], f32, name="rot")
        t1 = p_rope.tile([P, QS], f32, name="t1")
        for b0 in range(0, part_hi, ROPE):
            nc.vector.tensor_scalar_mul(
                out=rot[b0:b0 + HALF], in0=src[b0 + HALF:b0 + ROPE],
                scalar1=-1.0)
            nc.vector.tensor_copy(
                out=rot[b0 + HALF:b0 + ROPE], in_=src[b0:b0 + HALF])
        csl = cos_sb[:part_hi, q0:q0 + QS]
        ssl = sin_sb[:part_hi, q0:q0 + QS]
        nc.vector.tensor_mul(t1[:part_hi], src, csl)
        nc.vector.tensor_mul(rot[:part_hi], rot[:part_hi], ssl)
        for dst in dsts:
            nc.vector.tensor_add(dst, t1[:part_hi], rot[:part_hi])

    def emit_A_super(sc, hq):
        q0 = sc * QS
        for group in GROUPS:
            ps = psA.tile([P, 2, QS], f32, name="psA")
            for k in range(NKC):
                rhs = hq[k // 8][:, k % 8, :]
                for i, (kind, idx) in enumerate(group):
                    c0, cw = mcol(kind, idx)
                    nc.tensor.matmul(
                        ps[:cw, i, :], wa_sb[:, k, c0:c0 + cw], rhs,
                        start=(k == 0), stop=(k == NKC - 1))
            for i, (kind, idx) in enumerate(group):
                if kind == "qn":
                    nc.scalar.copy(out=QTn[:, idx, q0:q0 + QS],
                                   in_=ps[:, i, :])
                elif kind == "pe":
                    rope_psum(ps[:, i, :], P, q0,
                              [QTp[:, idx, q0:q0 + QS]])
                elif kind == "kpe":
                    rope_psum(ps[:ROPE, i, :], ROPE, q0,
                              [kpeT[:ROPE, q0:q0 + QS],
                               kpeT[ROPE:, q0:q0 + QS]])
                else:  # this core's ckv chunk -> HBM -> AllGather
                    own = p_ckvo.tile([P, QS], bf16, name="ckv_own")
                    nc.scalar.copy(out=own, in_=ps[:, i, :])
                    # ACT queue: lands right behind the drain, skipping the
                    # SP queue where the big wa/hq loads serialize (~3us
                    # of descriptor generation each).
                    nc.scalar.dma_start(out=ccp[sc], in_=own)
                    nc.gpsimd.collective_compute(
                        "AllGather", mybir.AluOpType.bypass,
                        replica_groups=CC_GROUPS, ins=[ccp[sc]],
                        outs=[ccg[sc]])
                    for rc in range(RC):
                        # SP queue: idle after startup, and keeping these
                        # off gpsimd stops their descriptor generation from
                        # delaying the next super's gather.
                        nc.sync.dma_start(out=ckvT[:, rc, q0:q0 + QS],
                                          in_=ccg[sc, rc])

    def emit_B_super(sc, pools, tag="psb"):
        b_sq, b_rs, b_psS, b_psB = pools
        q0 = sc * QS
        # RMS over the gathered full latent: sumsq via ones-matmul,
        # rsqrt broadcast already landed across partitions by the matmul.
        ps_ss = b_psS.tile([P, QS], f32, name=tag)
        for rc in range(RC):
            sq = b_sq.tile([P, QS], fr, name="sq")
            nc.scalar.activation(out=sq, in_=ckvT[:, rc, q0:q0 + QS],
                                 func=AF.Square)
            nc.tensor.matmul(ps_ss, ones_pp, sq,
                             start=(rc == 0), stop=(rc == RC - 1))
        s_b = b_rs.tile([P, QS], f32, name="s_b")
        nc.scalar.activation(out=s_b, in_=ps_ss, func=AF.Sqrt,
                             bias=eps_sb, scale=1.0 / KV_RANK)
        rs_b = b_rs.tile([P, QS], f32, name="rs_b")
        nc.vector.reciprocal_approx_fast(out=rs_b, in_=s_b)
        for rc in range(RC):
            nc.vector.tensor_mul(ckvT[:, rc, q0:q0 + QS],
                                 ckvT[:, rc, q0:q0 + QS], rs_b)
        for h in range(HPC):
            ps = b_psB.tile([P, QS], f32, name=tag)
            for rc in range(RC):
                nc.tensor.matmul(
                    ps, kvbk_sb[:, rc, h * NOPE:(h + 1) * NOPE],
                    ckvT[:, rc, sc * QS:(sc + 1) * QS],
                    start=(rc == 0), stop=(rc == RC - 1))
            nc.scalar.copy(out=KTn[:, h, sc * QS:(sc + 1) * QS], in_=ps)
        for stl in range(4):
            st = sc * 4 + stl
            psv = b_psB.tile([P, QS], f32, name=tag)
            for rc in range(RC):
                nc.tensor.matmul(
                    psv, ckvT[:, rc, st * P:(st + 1) * P],
                    kvbv_sb[:, rc, :],
                    start=(rc == 0), stop=(rc == RC - 1))
            nc.scalar.copy(out=Vsb[:, st, :, :],
                           in_=psv.rearrange("p (h v) -> p h v", h=HPC))

    for sc in range(NQ):
        hq = pend.pop(0)
        if sc + 3 < NQ:
            pend.append([hq_dma(sc + 3, 0), hq_dma(sc + 3, 1)])
        emit_A_super(sc, hq)
        if sc >= 1:
            emit_B_super(sc - 1, (p_sq, p_rs, psS, psB))

    for cm in reversed(p_A_cms):
        cm.__exit__(None, None, None)
    p_cs_cm.__exit__(None, None, None)  # free cos/sin

    p_ow_cm = tc.tile_pool(name="ow" + sfx, bufs=1)
    p_ow = p_ow_cm.__enter__()
    owT_sb = p_ow.tile([P, HPC, D], bf16, name="owT")
    p_at_cm = tc.tile_pool(name="at" + sfx, bufs=1)
    p_at = p_at_cm.__enter__()
    at = p_at.tile([P, HPC, S], bf16, name="at")
    for hc in range(HPC):
        nc.sync.dma_start(out=owT_sb[:, hc, :], in_=owT[:, hc, :])

    # ============ Phase C: attention + o_proj, per q-super ================
    # B(3) is emitted behind C(qt=0) so the last gather hides under
    # attention.  PSUM: psSC 2 + psPV 3 + psM 1 + (psB 2 until B(3) done,
    # then psD 2) = 8 banks.
    with tc.tile_pool(name="Ce" + sfx, bufs=4) as pEP, \
         tc.tile_pool(name="Ca" + sfx, bufs=2) as pEacc, \
         tc.tile_pool(name="Cr" + sfx, bufs=2) as pRec, \
         tc.tile_pool(name="psC" + sfx, bufs=2, space="PSUM") as psSC, \
         tc.tile_pool(name="psP" + sfx, bufs=3, space="PSUM") as psPV, \
         tc.tile_pool(name="psM" + sfx, bufs=1, space="PSUM") as psM:

        def emit_attn_super(qt, post=None):
            q0 = qt * QS
            nj = 4 * qt + 4
            for h in range(HPC):
                hp = (h % 2) * ROPE
                qprhs = QTp[hp:hp + ROPE, h // 2, :]
                E_acc = pEacc.tile([P, QS], fr, name="E_acc")
                ps_pv = psPV.tile([P, QS], f32, name="ps_pv")
                for j in range(nj):
                    jl = j - 4 * qt
                    off = max(jl, 0) * P
                    ps_sc = psSC.tile([P, QS], f32, name="ps_sc")
                    nc.tensor.matmul(
                        ps_sc[:, off:], KTn[:, h, j * P:(j + 1) * P],
                        QTn[:, h, q0 + off:q0 + QS], start=True, stop=False)
                    nc.tensor.matmul(
                        ps_sc[:, off:],
                        kpeT[hp:hp + ROPE, j * P:(j + 1) * P],
                        qprhs[:, q0 + off:q0 + QS], start=False, stop=True)
                    ep = pEP.tile([P, QS], bf16, name="ep")
                    nc.scalar.activation(out=ep[:, off:], in_=ps_sc[:, off:],
                                         func=AF.Exp)
                    if jl >= 0:  # diagonal super-block: causal mask
                        nc.vector.tensor_mul(ep[:, off:], ep[:, off:],
                                             mask_sb[:, jl, off:])
                    if j == 0:
                        nc.vector.tensor_copy(out=E_acc, in_=ep)
                    else:
                        nc.vector.tensor_add(E_acc[:, off:], E_acc[:, off:],
                                             ep[:, off:])
                    nc.tensor.matmul(ps_pv[:, off:], Vsb[:, j, h, :],
                                     ep[:, off:], start=(j == 0),
                                     stop=(jl == 3))
                # broadcast column sums on PE, 1/x on DVE, fused drain
                ps_sums = psM.tile([P, QS], f32, name="ps_sums")
                nc.tensor.matmul(ps_sums, ones_pp, E_acc, start=True, stop=True)
                rec = pRec.tile([P, QS], f32, name="rec")
                nc.vector.reciprocal_approx_fast(out=rec, in_=ps_sums)
                nc.vector.tensor_mul(at[:, h, q0:q0 + QS], ps_pv, rec)
                if post is not None:
                    post(h)

        # C(qt=0) first (needs only B(0)); B(3) is emitted after C(qt=1)
        # sharing psD's bank pair (same tile tag), so the last gather hides
        # under ~35us of attention work.
        emit_attn_super(0)
        with tc.tile_pool(name="B3sq" + sfx, bufs=2) as b_sq, \
             tc.tile_pool(name="B3rs" + sfx, bufs=2) as b_rs, \
             tc.tile_pool(name="Dy" + sfx, bufs=4) as pDy, \
             tc.tile_pool(name="psD" + sfx, bufs=2, space="PSUM") as psD:

            def emit_oproj_st(qt, stl):
                st = qt * 4 + stl
                for nk in range(NQ):
                    psy = psD.tile([P, QS], f32, name="psy")
                    for hc in range(HPC):
                        nc.tensor.matmul(
                            psy, at[:, hc, st * P:(st + 1) * P],
                            owT_sb[:, hc, nk * QS:(nk + 1) * QS],
                            start=(hc == 0), stop=(hc == HPC - 1))
                    ys = pDy.tile([P, QS], bf16, name="ys")
                    nc.scalar.copy(out=ys, in_=psy)
                    nc.gpsimd.dma_start(
                        out=y[st * P:(st + 1) * P, nk * QS:(nk + 1) * QS],
                        in_=ys)

            for qt in range(1, NQ):
                emit_attn_super(qt, post=lambda h, q=qt: emit_oproj_st(q - 1, h))
                if qt == 1:
                    emit_B_super(NQ - 1, (b_sq, b_rs, psD, psD), tag="psy")
            for stl in range(4):
                emit_oproj_st(NQ - 1, stl)

    p_at_cm.__exit__(None, None, None)
    p_ow_cm.__exit__(None, None, None)
    p_ckv_cm.__exit__(None, None, None)
    p_bw_cm.__exit__(None, None, None)
    p_kvb_cm.__exit__(None, None, None)
    p_qt_cm.__exit__(None, None, None)
    p_const_cm.__exit__(None, None, None)


def _build_program(reps=1):
    import concourse.bacc as bacc
    import concourse.tile as tile

    nc = bacc.Bacc("TRN2", target_bir_lowering=False, debug=False,
                   num_devices=NCORES)
    with tile.TileContext(nc) as tc:
        io = _declare_io(nc)
        for r in range(reps):
            _emit(nc, tc, io, sfx=f"_r{r}" if reps > 1 else "")
    nc.compile()
    return nc


def _rope_cos_sin():
    inv_freq = 1.0 / (BASE ** (np.arange(0, ROPE, 2, dtype=np.float32) / ROPE))
    t = np.arange(S, dtype=np.float32)
    freqs = np.outer(t, inv_freq)                     # [S, ROPE/2]
    emb = np.concatenate([freqs, freqs], axis=-1)     # [S, ROPE]
    return np.cos(emb), np.sin(emb)


def _host_prep(hidden_states, q_proj_w, kv_a_proj_w, kv_a_norm_w,
               kv_b_proj_w, o_proj_w):
    """Build per-core input maps (bf16 operands)."""
    import ml_dtypes

    bf = ml_dtypes.bfloat16
    hidden_states = np.asarray(hidden_states, dtype=np.float32)
    q_proj_w = np.asarray(q_proj_w, dtype=np.float32)
    kv_a_proj_w = np.asarray(kv_a_proj_w, dtype=np.float32)
    kv_a_norm_w = np.asarray(kv_a_norm_w, dtype=np.float32)
    kv_b_proj_w = np.asarray(kv_b_proj_w, dtype=np.float32)
    o_proj_w = np.asarray(o_proj_w, dtype=np.float32)

    scale = np.float32(1.0 / math.sqrt(QHD))
    qws = (q_proj_w * scale).reshape(H, QHD, D)
    kvb = (kv_b_proj_w * kv_a_norm_w[None, :]).reshape(H, NOPE + VD, KV_RANK)

    cos, sin = _rope_cos_sin()                             # [S, ROPE]
    cosT2 = np.ascontiguousarray(np.tile(cos.T, (2, 1)))   # [128, S] f32
    sinT2 = np.ascontiguousarray(np.tile(sin.T, (2, 1)))

    # diag masks, stored partition-major: masks[p, j, q]
    r = np.arange(P)[:, None]
    ql = np.arange(QS)[None, :]
    masks = np.stack([(ql >= j * P + r).astype(np.float32) for j in range(4)])
    masks = np.ascontiguousarray(masks.transpose(1, 0, 2)).astype(bf)

    in_maps = []
    for c in range(NCORES):
        b, g = divmod(c, HPC)
        heads = list(range(HPC * g, HPC * g + HPC))
        hT = np.ascontiguousarray(hidden_states[b].T).astype(bf)   # [D, S]
        # wA cols: 4x nope(128), 2x pe-pair(128), kpe-w(64), ckv-chunk(128)
        cols = np.concatenate(
            [qws[h, :NOPE, :] for h in heads]
            + [qws[h, NOPE:, :] for h in heads]
            + [kv_a_proj_w[KV_RANK:, :],
               kv_a_proj_w[g * P:(g + 1) * P, :]],
            axis=0)                                        # [960, D]
        wA_full = cols.T                                   # [D, 960]
        # flat, block-contiguous in WBLOCKS order (device DMAs per block)
        wA_c = np.concatenate(
            [np.ascontiguousarray(wA_full[:, c0:c0 + cw]).ravel()
             for c0, cw in WBLOCKS]).astype(bf)
        # kvbk [128, 4, 4*128]: kvbk[p, rc, h*128+j] = kvb[heads[h], j, rc*128+p]
        kn = np.stack([kvb[h, :NOPE, :] for h in heads])    # [h, j, r]
        kvbk_c = np.ascontiguousarray(
            kn.transpose(2, 0, 1).reshape(RC, P, HPC, NOPE)
            .transpose(1, 0, 2, 3).reshape(P, RC, HPC * NOPE)).astype(bf)
        kv = np.stack([kvb[h, NOPE:, :] for h in heads])    # [h, j(vd), r]
        kvbv_c = np.ascontiguousarray(
            kv.transpose(2, 0, 1).reshape(RC, P, HPC, VD)
            .transpose(1, 0, 2, 3).reshape(P, RC, HPC * VD)).astype(bf)
        # owT [128, 4, D]: owT[p, hc, n] = o_proj_w[n, g*512 + hc*128 + p]
        ow = o_proj_w[:, g * HPC * VD:(g + 1) * HPC * VD]   # [D, 512]
        owT_c = np.ascontiguousarray(
            ow.T.reshape(HPC, VD, D).transpose(1, 0, 2)).astype(bf)
        in_maps.append({
            "hT": hT, "wA": wA_c,
            "kvbk": kvbk_c, "kvbv": kvbv_c, "owT": owT_c,
            "cosT2": cosT2, "sinT2": sinT2, "masks": masks,
        })
    return in_maps


def _gather(results):
    out = np.zeros((B, S, D), dtype=np.float32)
    for c in range(NCORES):
        out[c // HPC] += np.asarray(results[c]["y"], dtype=np.float32)
    return out


def kernel(hidden_states, q_proj_w, kv_a_proj_w, kv_a_norm_w,
           kv_b_proj_w, o_proj_w):
    from concourse import bass_utils

    in_maps = _host_prep(hidden_states, q_proj_w, kv_a_proj_w, kv_a_norm_w,
                         kv_b_proj_w, o_proj_w)
    if "nc" not in _CACHE:
        _CACHE["nc"] = _build_program()
    nc = _CACHE["nc"]
    res = bass_utils.run_bass_kernel_spmd(nc, in_maps, list(range(NCORES)))
    return _gather(res.results)


if __name__ == "__main__":
    rng = np.random.default_rng(0)
    ins = {
        "hidden_states": rng.standard_normal((B, S, D), dtype=np.float32),
        "q_proj_w": rng.standard_normal((H * QHD, D), dtype=np.float32) * D ** -0.5,
        "kv_a_proj_w": rng.standard_normal((KV_RANK + ROPE, D), dtype=np.float32) * D ** -0.5,
        "kv_a_norm_w": np.ones(KV_RANK, dtype=np.float32),
        "kv_b_proj_w": rng.standard_normal((H * (NOPE + VD), KV_RANK), dtype=np.float32) * KV_RANK ** -0.5,
        "o_proj_w": rng.standard_normal((D, H * VD), dtype=np.float32) * (H * VD) ** -0.5,
    }
    out = kernel(**ins)
    print(out.shape, out.dtype, float(np.abs(out).mean()))


# revision 24
# speedup vs baseline: 62.2911x; 1.0449x over previous
"""Trainium2 Bass kernel for DeepSeek-style MLA (multi-head latent attention).

Sharding: 8 cores = 2 (batch) x 4 (head-groups of 4 heads).
Core c handles batch b = c // 4 and heads [4*(c%4), 4*(c%4)+4).
Each core computes its 4 heads' full attention + its partial o_proj
contribution y_partial [S, D] (bf16); host sums the 4 partials per batch.

v3 design (vs v2 baseline at ~295us):
  - Latent (ckv) projection sharded 4-way across the cores of each batch
    group: core c computes only ckv rows [128*(c%4), 128*(c%4)+128) (the
    chunk is baked into its wA input data; the program stays SPMD-pure).
    Per q-super, chunks are exchanged with an HBM AllGather over replica
    groups [[0..3],[4..7]] and read back in global chunk order, so no
    instruction depends on the core id.  This removes 4.6 GFLOP/core of
    redundant projection work (A phase: 11 m-tiles -> 8 per super).
  - RMS norm moved after the gather (sum-of-squares via ones-matmul over
    the 4 gathered chunks, rsqrt on ACT, scale on DVE) inside phase B.
  - Phase C starts with q-super 0 (needs only B(0)) and B(3) is emitted
    inside phase C behind C(qt=0), hiding the last gather's latency.
  - Everything else follows v2: bf16 matmuls with fp32 PSUM, partition
    broadcasts via all-ones lhsT matmuls, exact 128-granular causal
    narrowing, o_proj interleaved one q-super behind attention.
"""

import math
import sys

import numpy as np

for _p in ("/opt/trn_rl_repo",):
    if _p not in sys.path:
        sys.path.insert(0, _p)

# ---- problem constants (hardcoded per contract) ----
B = 2
S = 2048
D = 2048
H = 16
NOPE = 128
ROPE = 64
VD = 128
KV_RANK = 512
QHD = NOPE + ROPE
EPS = 1e-6
BASE = 10000.0

HPC = 4            # heads per core
NCORES = 8
P = 128
QS = 512           # q-super width
NQ = S // QS       # 4
NST = S // P       # 16 s-tiles
NKC = D // P       # 16 d-chunks
RC = KV_RANK // P  # 4 r-chunks
HALF = ROPE // 2   # 32

# wA column layout: 4x q-nope(128) | 2x q-pe pair(128) | kpe-w(64) | ckv-chunk(128)
WQ = HPC * QHD       # 768 (q cols)
WKPE = WQ            # 768, offset of kpe cols
WCKV = WQ + ROPE     # 832, offset of this core's ckv chunk cols
WCOLS = WCKV + P     # 960
# contiguous weight DMA blocks (col offset, width), in consumption order
WBLOCKS = [(0, 256), (256, 256), (512, 256), (768, 192)]

CC_GROUPS = [[0, 1, 2, 3], [4, 5, 6, 7]]

_CACHE = {}


def _declare_io(nc):
    import concourse.mybir as mybir

    f32 = mybir.dt.float32
    bf16 = mybir.dt.bfloat16
    io = {}
    io["hT"] = nc.dram_tensor("hT", [D, S], bf16, kind="ExternalInput").ap()
    io["wA"] = nc.dram_tensor("wA", [D * WCOLS], bf16, kind="ExternalInput").ap()
    io["kvbk"] = nc.dram_tensor("kvbk", [P, RC, HPC * NOPE], bf16, kind="ExternalInput").ap()
    io["kvbv"] = nc.dram_tensor("kvbv", [P, RC, HPC * VD], bf16, kind="ExternalInput").ap()
    io["owT"] = nc.dram_tensor("owT", [P, HPC, D], bf16, kind="ExternalInput").ap()
    io["cosT2"] = nc.dram_tensor("cosT2", [P, S], bf16, kind="ExternalInput").ap()
    io["sinT2"] = nc.dram_tensor("sinT2", [P, S], bf16, kind="ExternalInput").ap()
    io["masks"] = nc.dram_tensor("masks", [P, 4, QS], bf16, kind="ExternalInput").ap()
    io["y"] = nc.dram_tensor("y", [S, D], bf16, kind="ExternalOutput").ap()
    return io


def _declare_cc(nc, sfx=""):
    import concourse.mybir as mybir

    bf16 = mybir.dt.bfloat16
    part = nc.dram_tensor("ckv_part" + sfx, [NQ, P, QS], bf16, kind="Internal").ap()
    gath = nc.dram_tensor("ckv_gath" + sfx, [NQ, RC, P, QS], bf16, kind="Internal").ap()
    return part, gath


def _emit(nc, tc, io, sfx=""):
    """Emit the whole per-core program into TileContext tc."""
    import concourse.mybir as mybir

    f32 = mybir.dt.float32
    fr = mybir.dt.float32r
    bf16 = mybir.dt.bfloat16
    AF = mybir.ActivationFunctionType

    hT = io["hT"]; wA = io["wA"]
    kvbk = io["kvbk"]; kvbv = io["kvbv"]; owT = io["owT"]
    cosT2 = io["cosT2"]; sinT2 = io["sinT2"]; masks = io["masks"]
    y = io["y"]
    ccp, ccg = _declare_cc(nc, sfx)

    # ---- long-lived pools, strictly nested (LIFO exit) ----
    p_const_cm = tc.tile_pool(name="const" + sfx, bufs=1)
    p_const = p_const_cm.__enter__()
    p_qt_cm = tc.tile_pool(name="qt" + sfx, bufs=1)        # QTn/QTp/kpeT
    p_qt = p_qt_cm.__enter__()
    p_kvb_cm = tc.tile_pool(name="kvb" + sfx, bufs=1)      # KTn/Vsb: B -> attn
    p_kvb = p_kvb_cm.__enter__()
    p_bw_cm = tc.tile_pool(name="Bw" + sfx, bufs=1)        # kv_b weights
    p_bw = p_bw_cm.__enter__()
    p_ckv_cm = tc.tile_pool(name="ckv" + sfx, bufs=1)      # ckvT: gather -> B
    p_ckv = p_ckv_cm.__enter__()
    p_cs_cm = tc.tile_pool(name="cossin" + sfx, bufs=1)    # cos/sin: A only
    p_cs = p_cs_cm.__enter__()

    ones_pp_f = p_const.tile([P, P], f32, name="ones_pp")
    nc.vector.memset(ones_pp_f, 1.0)
    ones_pp = ones_pp_f.bitcast(fr)
    eps_sb = p_const.tile([P, 1], f32, name="eps")
    nc.vector.memset(eps_sb, EPS)
    mask_sb = p_const.tile([P, 4, QS], bf16, name="masks")

    QTn = p_qt.tile([P, HPC, S], bf16, name="QTn")
    QTp = p_qt.tile([P, 2, S], bf16, name="QTp")
    # kpeT duplicated on partitions [0:64] and [64:128] (lhsT base must
    # match rhs base per head parity)
    kpeT = p_qt.tile([P, S], bf16, name="kpeT")
    ckvT = p_ckv.tile([P, RC, S], bf16, name="ckvT")
    KTn = p_kvb.tile([P, HPC, S], bf16, name="KTn")
    Vsb = p_kvb.tile([P, NST, HPC, VD], bf16, name="Vsb")
    kvbk_sb = p_bw.tile([P, RC, HPC * NOPE], bf16, name="kvbk")
    kvbv_sb = p_bw.tile([P, RC, HPC * VD], bf16, name="kvbv")
    cos_sb = p_cs.tile([P, S], bf16, name="cos")
    sin_sb = p_cs.tile([P, S], bf16, name="sin")

    # ============ Phase A: projections, one super at a time =============
    # A groups (pairs of m-tiles, 2 banks each): [qn0,qn1] [qn2,qn3]
    # [pe0,pe1] [kpe,ckv_own].  PSUM: psA 2x2 + stats 1 + psB 3 = 8 banks.
    # After each super's A groups, the core's ckv chunk is pushed to HBM
    # and AllGather'ed; phase B for super sc is emitted after super sc+1's
    # A groups so the gather latency hides behind projection work.  B(3)
    # is emitted later, behind C(qt=0), with its own nested pools.
    p_A_cms = [tc.tile_pool(name="Ah" + sfx, bufs=6),
               tc.tile_pool(name="Aw" + sfx, bufs=1),
               tc.tile_pool(name="Ar" + sfx, bufs=2),
               tc.tile_pool(name="Ack" + sfx, bufs=2),
               tc.tile_pool(name="Bsq" + sfx, bufs=2),
               tc.tile_pool(name="Brs" + sfx, bufs=2),
               tc.tile_pool(name="psA" + sfx, bufs=2, space="PSUM"),
               tc.tile_pool(name="psS" + sfx, bufs=1, space="PSUM"),
               tc.tile_pool(name="psB" + sfx, bufs=3, space="PSUM")]
    p_hq, p_wa, p_rope, p_ckvo, p_sq, p_rs, psA, psS, psB = [
        cm.__enter__() for cm in p_A_cms]

    wa_sb = p_wa.tile([P, NKC, WCOLS], bf16, name="wa")

    def wa_dma(b, half=None, eng=None):
        eng = eng or nc.sync
        c0, cw = WBLOCKS[b]
        off = D * c0
        src_ = wA[off:off + D * cw].rearrange(
            "(kk p c) -> p kk c", p=P, c=cw)
        if half is None:
            eng.dma_start(out=wa_sb[:, :, c0:c0 + cw], in_=src_)
        else:
            k0, k1 = (0, NKC // 2) if half == 0 else (NKC // 2, NKC)
            eng.dma_start(out=wa_sb[:, k0:k1, c0:c0 + cw],
                          in_=src_[:, k0:k1, :])

    def hq_dma(sc, half, split=False, eng=None):
        eng = eng or nc.sync
        t = p_hq.tile([P, NKC // 2, QS], bf16, name="hq")
        src = hT[half * 1024:(half + 1) * 1024,
                 sc * QS:(sc + 1) * QS].rearrange("(kk p) s -> p kk s", p=P)
        if split:  # startup: land the first k-chunks earlier
            nc.sync.dma_start(out=t[:, 0:4, :], in_=src[:, 0:4, :])
            nc.sync.dma_start(out=t[:, 4:8, :], in_=src[:, 4:8, :])
        else:
            eng.dma_start(out=t, in_=src)
        return t

    # startup, all on SP in consumption order ([kpe,ckv] group first: wa
    # block 3, hq, cos/sin).  Keeping startup on SP lets the NEXT rep's
    # loads prefetch during this rep's attention phase (SP idles there),
    # so back-to-back reps pipeline.
    wa_dma(3)
    pend = [[hq_dma(0, 0, split=True)]]
    pend[0].append(hq_dma(0, 1))
    nc.sync.dma_start(out=cos_sb, in_=cosT2)
    nc.sync.dma_start(out=sin_sb, in_=sinT2)
    wa_dma(0, half=0)
    wa_dma(0, half=1)
    wa_dma(1)
    pend.append([hq_dma(1, 0)])
    wa_dma(2)
    pend[1].append(hq_dma(1, 1))
    pend.append([hq_dma(2, 0), hq_dma(2, 1)])
    # prefetch phase-B weights + masks during A
    for rc in range(RC):
        nc.sync.dma_start(out=kvbk_sb[:, rc, :], in_=kvbk[:, rc, :])
        nc.sync.dma_start(out=kvbv_sb[:, rc, :], in_=kvbv[:, rc, :])
    nc.sync.dma_start(out=mask_sb, in_=masks)

    # m-tile groups: (kind, idx); [kpe, ckv] first so the gather starts early
    GROUPS = [
        [("kpe", 0), ("ckv", 0)],
        [("qn", 0), ("qn", 1)],
        [("qn", 2), ("qn", 3)],
        [("pe", 0), ("pe", 1)],
    ]

    def mcol(kind, idx):
        if kind == "qn":
            return idx * P, P
        if kind == "pe":
            return (4 + idx) * P, P
        if kind == "kpe":
            return WKPE, ROPE
        return WCKV, P

    def rope_psum(src, part_hi, q0, dsts):
        """RoPE from PSUM src [part_hi, QS] -> each dst slice (bf16).
        cos/sin rows repeat every 64 partitions."""
        rot = p_rope.tile([P, QS], f32, name="rot")
        t1 = p_rope.tile([P, QS], f32, name="t1")
        for b0 in range(0, part_hi, ROPE):
            nc.vector.tensor_scalar_mul(
                out=rot[b0:b0 + HALF], in0=src[b0 + HALF:b0 + ROPE],
                scalar1=-1.0)
            nc.vector.tensor_copy(
                out=rot[b0 + HALF:b0 + ROPE], in_=src[b0:b0 + HALF])
        csl = cos_sb[:part_hi, q0:q0 + QS]
        ssl = sin_sb[:part_hi, q0:q0 + QS]
        nc.vector.tensor_mul(t1[:part_hi], src, csl)
        nc.vector.tensor_mul(rot[:part_hi], rot[:part_hi], ssl)
        for dst in dsts:
            nc.vector.tensor_add(dst, t1[:part_hi], rot[:part_hi])

    def emit_A_group(sc, hq, group):
        q0 = sc * QS
        ps = psA.tile([P, 2, QS], f32, name="psA")
        for k in range(NKC):
            rhs = hq[k // 8][:, k % 8, :]
            for i, (kind, idx) in enumerate(group):
                c0, cw = mcol(kind, idx)
                nc.tensor.matmul(
                    ps[:cw, i, :], wa_sb[:, k, c0:c0 + cw], rhs,
                    start=(k == 0), stop=(k == NKC - 1))
        for i, (kind, idx) in enumerate(group):
            if kind == "qn":
                nc.scalar.copy(out=QTn[:, idx, q0:q0 + QS],
                               in_=ps[:, i, :])
            elif kind == "pe":
                rope_psum(ps[:, i, :], P, q0,
                          [QTp[:, idx, q0:q0 + QS]])
            elif kind == "kpe":
                rope_psum(ps[:ROPE, i, :], ROPE, q0,
                          [kpeT[:ROPE, q0:q0 + QS],
                           kpeT[ROPE:, q0:q0 + QS]])
            else:  # this core's ckv chunk -> HBM -> AllGather
                own = p_ckvo.tile([P, QS], bf16, name="ckv_own")
                nc.scalar.copy(out=own, in_=ps[:, i, :])
                # gpsimd queue: right before its collective, keeping
                # descriptor generation off the SP/ACT queues.
                nc.gpsimd.dma_start(out=ccp[sc], in_=own)
                nc.gpsimd.collective_compute(
                    "AllGather", mybir.AluOpType.bypass,
                    replica_groups=CC_GROUPS, ins=[ccp[sc]],
                    outs=[ccg[sc]])
                for rc in range(RC):
                    # SP queue: idle after startup, and keeping these
                    # off gpsimd stops their descriptor generation from
                    # delaying the next super's gather.
                    nc.sync.dma_start(out=ckvT[:, rc, q0:q0 + QS],
                                      in_=ccg[sc, rc])

    def emit_B_super(sc, pools, tag="psb"):
        b_sq, b_rs, b_psS, b_psB = pools
        q0 = sc * QS
        # RMS over the gathered full latent: sumsq via ones-matmul,
        # rsqrt broadcast already landed across partitions by the matmul.
        ps_ss = b_psS.tile([P, QS], f32, name=tag)
        for rc in range(RC):
            sq = b_sq.tile([P, QS], fr, name="sq")
            nc.scalar.activation(out=sq, in_=ckvT[:, rc, q0:q0 + QS],
                                 func=AF.Square)
            nc.tensor.matmul(ps_ss, ones_pp, sq,
                             start=(rc == 0), stop=(rc == RC - 1))
        s_b = b_rs.tile([P, QS], f32, name="s_b")
        nc.scalar.activation(out=s_b, in_=ps_ss, func=AF.Sqrt,
                             bias=eps_sb, scale=1.0 / KV_RANK)
        rs_b = b_rs.tile([P, QS], f32, name="rs_b")
        nc.vector.reciprocal_approx_fast(out=rs_b, in_=s_b)
        for rc in range(RC):
            nc.vector.tensor_mul(ckvT[:, rc, q0:q0 + QS],
                                 ckvT[:, rc, q0:q0 + QS], rs_b)
        for h in range(HPC):
            ps = b_psB.tile([P, QS], f32, name=tag)
            for rc in range(RC):
                nc.tensor.matmul(
                    ps, kvbk_sb[:, rc, h * NOPE:(h + 1) * NOPE],
                    ckvT[:, rc, sc * QS:(sc + 1) * QS],
                    start=(rc == 0), stop=(rc == RC - 1))
            nc.scalar.copy(out=KTn[:, h, sc * QS:(sc + 1) * QS], in_=ps)
        for stl in range(4):
            st = sc * 4 + stl
            psv = b_psB.tile([P, QS], f32, name=tag)
            for rc in range(RC):
                nc.tensor.matmul(
                    psv, ckvT[:, rc, st * P:(st + 1) * P],
                    kvbv_sb[:, rc, :],
                    start=(rc == 0), stop=(rc == RC - 1))
            nc.scalar.copy(out=Vsb[:, st, :, :],
                           in_=psv.rearrange("p (h v) -> p h v", h=HPC))

    # ckv groups lead the q groups by ~2 supers, so each AllGather has
    # ~2 supers (~55us) of projection work to hide behind before its
    # B phase consumes the gathered chunks.
    hq = {sc: pend[sc] for sc in range(3)}
    bpools = (p_sq, p_rs, psS, psB)
    emit_A_group(0, hq[0], GROUPS[0])
    emit_A_group(1, hq[1], GROUPS[0])
    hq[3] = [hq_dma(3, 0), hq_dma(3, 1)]
    for g in (1, 2, 3):
        emit_A_group(0, hq[0], GROUPS[g])
    emit_A_group(2, hq[2], GROUPS[0])
    for g in (1, 2, 3):
        emit_A_group(1, hq[1], GROUPS[g])
    emit_B_super(0, bpools)
    emit_A_group(3, hq[3], GROUPS[0])
    for g in (1, 2, 3):
        emit_A_group(2, hq[2], GROUPS[g])
    emit_B_super(1, bpools)
    for g in (1, 2, 3):
        emit_A_group(3, hq[3], GROUPS[g])
    emit_B_super(2, bpools)

    for cm in reversed(p_A_cms):
        cm.__exit__(None, None, None)
    p_cs_cm.__exit__(None, None, None)  # free cos/sin

    p_ow_cm = tc.tile_pool(name="ow" + sfx, bufs=1)
    p_ow = p_ow_cm.__enter__()
    owT_sb = p_ow.tile([P, HPC, D], bf16, name="owT")
    p_at_cm = tc.tile_pool(name="at" + sfx, bufs=1)
    p_at = p_at_cm.__enter__()
    at = p_at.tile([P, HPC, S], bf16, name="at")
    for hc in range(HPC):
        nc.sync.dma_start(out=owT_sb[:, hc, :], in_=owT[:, hc, :])

    # ============ Phase C: attention + o_proj, per q-super ================
    # B(3) is emitted behind C(qt=0) so the last gather hides under
    # attention.  PSUM: psSC 2 + psPV 3 + psM 1 + (psB 2 until B(3) done,
    # then psD 2) = 8 banks.
    with tc.tile_pool(name="Ce" + sfx, bufs=4) as pEP, \
         tc.tile_pool(name="Ca" + sfx, bufs=2) as pEacc, \
         tc.tile_pool(name="Cr" + sfx, bufs=2) as pRec, \
         tc.tile_pool(name="psC" + sfx, bufs=2, space="PSUM") as psSC, \
         tc.tile_pool(name="psP" + sfx, bufs=3, space="PSUM") as psPV, \
         tc.tile_pool(name="psM" + sfx, bufs=1, space="PSUM") as psM:

        def emit_attn_super(qt, post=None):
            q0 = qt * QS
            nj = 4 * qt + 4
            for h in range(HPC):
                hp = (h % 2) * ROPE
                qprhs = QTp[hp:hp + ROPE, h // 2, :]
                E_acc = pEacc.tile([P, QS], fr, name="E_acc")
                ps_pv = psPV.tile([P, QS], f32, name="ps_pv")
                for j in range(nj):
                    jl = j - 4 * qt
                    off = max(jl, 0) * P
                    ps_sc = psSC.tile([P, QS], f32, name="ps_sc")
                    nc.tensor.matmul(
                        ps_sc[:, off:], KTn[:, h, j * P:(j + 1) * P],
                        QTn[:, h, q0 + off:q0 + QS], start=True, stop=False)
                    nc.tensor.matmul(
                        ps_sc[:, off:],
                        kpeT[hp:hp + ROPE, j * P:(j + 1) * P],
                        qprhs[:, q0 + off:q0 + QS], start=False, stop=True)
                    ep = pEP.tile([P, QS], bf16, name="ep")
                    nc.scalar.activation(out=ep[:, off:], in_=ps_sc[:, off:],
                                         func=AF.Exp)
                    if jl >= 0:  # diagonal super-block: causal mask
                        nc.vector.tensor_mul(ep[:, off:], ep[:, off:],
                                             mask_sb[:, jl, off:])
                    if j == 0:
                        nc.vector.tensor_copy(out=E_acc, in_=ep)
                    else:
                        nc.vector.tensor_add(E_acc[:, off:], E_acc[:, off:],
                                             ep[:, off:])
                    nc.tensor.matmul(ps_pv[:, off:], Vsb[:, j, h, :],
                                     ep[:, off:], start=(j == 0),
                                     stop=(jl == 3))
                # broadcast column sums on PE, 1/x on DVE, fused drain
                ps_sums = psM.tile([P, QS], f32, name="ps_sums")
                nc.tensor.matmul(ps_sums, ones_pp, E_acc, start=True, stop=True)
                rec = pRec.tile([P, QS], f32, name="rec")
                nc.vector.reciprocal_approx_fast(out=rec, in_=ps_sums)
                nc.vector.tensor_mul(at[:, h, q0:q0 + QS], ps_pv, rec)
                if post is not None:
                    post(h)

        # C(qt=0) first (needs only B(0)); B(3) is emitted after C(qt=1)
        # sharing psD's bank pair (same tile tag), so the last gather hides
        # under ~35us of attention work.
        emit_attn_super(0)
        with tc.tile_pool(name="B3sq" + sfx, bufs=2) as b_sq, \
             tc.tile_pool(name="B3rs" + sfx, bufs=2) as b_rs, \
             tc.tile_pool(name="Dy" + sfx, bufs=4) as pDy, \
             tc.tile_pool(name="psD" + sfx, bufs=2, space="PSUM") as psD:

            def emit_oproj_st(qt, stl):
                st = qt * 4 + stl
                for nk in range(NQ):
                    psy = psD.tile([P, QS], f32, name="psy")
                    for hc in range(HPC):
                        nc.tensor.matmul(
                            psy, at[:, hc, st * P:(st + 1) * P],
                            owT_sb[:, hc, nk * QS:(nk + 1) * QS],
                            start=(hc == 0), stop=(hc == HPC - 1))
                    ys = pDy.tile([P, QS], bf16, name="ys")
                    nc.scalar.copy(out=ys, in_=psy)
                    nc.gpsimd.dma_start(
                        out=y[st * P:(st + 1) * P, nk * QS:(nk + 1) * QS],
                        in_=ys)

            for qt in range(1, NQ):
                emit_attn_super(qt, post=lambda h, q=qt: emit_oproj_st(q - 1, h))
                if qt == 1:
                    emit_B_super(NQ - 1, (b_sq, b_rs, psD, psD), tag="psy")
            for stl in range(4):
                emit_oproj_st(NQ - 1, stl)

    p_at_cm.__exit__(None, None, None)
    p_ow_cm.__exit__(None, None, None)
    p_ckv_cm.__exit__(None, None, None)
    p_bw_cm.__exit__(None, None, None)
    p_kvb_cm.__exit__(None, None, None)
    p_qt_cm.__exit__(None, None, None)
    p_const_cm.__exit__(None, None, None)


def _build_program(reps=1):
    import concourse.bacc as bacc
    import concourse.tile as tile

    nc = bacc.Bacc("TRN2", target_bir_lowering=False, debug=False,
                   num_devices=NCORES)
    with tile.TileContext(nc) as tc:
        io = _declare_io(nc)
        for r in range(reps):
            _emit(nc, tc, io, sfx=f"_r{r}" if reps > 1 else "")
    nc.compile()
    return nc


def _rope_cos_sin():
    inv_freq = 1.0 / (BASE ** (np.arange(0, ROPE, 2, dtype=np.float32) / ROPE))
    t = np.arange(S, dtype=np.float32)
    freqs = np.outer(t, inv_freq)                     # [S, ROPE/2]
    emb = np.concatenate([freqs, freqs], axis=-1)     # [S, ROPE]
    return np.cos(emb), np.sin(emb)


def _host_prep(hidden_states, q_proj_w, kv_a_proj_w, kv_a_norm_w,
               kv_b_proj_w, o_proj_w):
    """Build per-core input maps (bf16 operands)."""
    import ml_dtypes

    bf = ml_dtypes.bfloat16
    hidden_states = np.asarray(hidden_states, dtype=np.float32)
    q_proj_w = np.asarray(q_proj_w, dtype=np.float32)
    kv_a_proj_w = np.asarray(kv_a_proj_w, dtype=np.float32)
    kv_a_norm_w = np.asarray(kv_a_norm_w, dtype=np.float32)
    kv_b_proj_w = np.asarray(kv_b_proj_w, dtype=np.float32)
    o_proj_w = np.asarray(o_proj_w, dtype=np.float32)

    scale = np.float32(1.0 / math.sqrt(QHD))
    qws = (q_proj_w * scale).reshape(H, QHD, D)
    kvb = (kv_b_proj_w * kv_a_norm_w[None, :]).reshape(H, NOPE + VD, KV_RANK)

    cos, sin = _rope_cos_sin()                             # [S, ROPE]
    cosT2 = np.ascontiguousarray(np.tile(cos.T, (2, 1))).astype(bf)  # [128, S]
    sinT2 = np.ascontiguousarray(np.tile(sin.T, (2, 1))).astype(bf)

    # diag masks, stored partition-major: masks[p, j, q]
    r = np.arange(P)[:, None]
    ql = np.arange(QS)[None, :]
    masks = np.stack([(ql >= j * P + r).astype(np.float32) for j in range(4)])
    masks = np.ascontiguousarray(masks.transpose(1, 0, 2)).astype(bf)

    in_maps = []
    for c in range(NCORES):
        b, g = divmod(c, HPC)
        heads = list(range(HPC * g, HPC * g + HPC))
        hT = np.ascontiguousarray(hidden_states[b].T).astype(bf)   # [D, S]
        # wA cols: 4x nope(128), 2x pe-pair(128), kpe-w(64), ckv-chunk(128)
        cols = np.concatenate(
            [qws[h, :NOPE, :] for h in heads]
            + [qws[h, NOPE:, :] for h in heads]
            + [kv_a_proj_w[KV_RANK:, :],
               kv_a_proj_w[g * P:(g + 1) * P, :]],
            axis=0)                                        # [960, D]
        wA_full = cols.T                                   # [D, 960]
        # flat, block-contiguous in WBLOCKS order (device DMAs per block)
        wA_c = np.concatenate(
            [np.ascontiguousarray(wA_full[:, c0:c0 + cw]).ravel()
             for c0, cw in WBLOCKS]).astype(bf)
        # kvbk [128, 4, 4*128]: kvbk[p, rc, h*128+j] = kvb[heads[h], j, rc*128+p]
        kn = np.stack([kvb[h, :NOPE, :] for h in heads])    # [h, j, r]
        kvbk_c = np.ascontiguousarray(
            kn.transpose(2, 0, 1).reshape(RC, P, HPC, NOPE)
            .transpose(1, 0, 2, 3).reshape(P, RC, HPC * NOPE)).astype(bf)
        kv = np.stack([kvb[h, NOPE:, :] for h in heads])    # [h, j(vd), r]
        kvbv_c = np.ascontiguousarray(
            kv.transpose(2, 0, 1).reshape(RC, P, HPC, VD)
            .transpose(1, 0, 2, 3).reshape(P, RC, HPC * VD)).astype(bf)
        # owT [128, 4, D]: owT[p, hc, n] = o_proj_w[n, g*512 + hc*128 + p]
        ow = o_proj_w[:, g * HPC * VD:(g + 1) * HPC * VD]   # [D, 512]
        owT_c = np.ascontiguousarray(
            ow.T.reshape(HPC, VD, D).transpose(1, 0, 2)).astype(bf)
        in_maps.append({
            "hT": hT, "wA": wA_c,
            "kvbk": kvbk_c, "kvbv": kvbv_c, "owT": owT_c,
            "cosT2": cosT2, "sinT2": sinT2, "masks": masks,
        })
    return in_maps


def _gather(results):
    out = np.zeros((B, S, D), dtype=np.float32)
    for c in range(NCORES):
        out[c // HPC] += np.asarray(results[c]["y"], dtype=np.float32)
    return out


def kernel(hidden_states, q_proj_w, kv_a_proj_w, kv_a_norm_w,
           kv_b_proj_w, o_proj_w):
    from concourse import bass_utils

    in_maps = _host_prep(hidden_states, q_proj_w, kv_a_proj_w, kv_a_norm_w,
                         kv_b_proj_w, o_proj_w)
    if "nc" not in _CACHE:
        _CACHE["nc"] = _build_program()
    nc = _CACHE["nc"]
    res = bass_utils.run_bass_kernel_spmd(nc, in_maps, list(range(NCORES)))
    return _gather(res.results)


if __name__ == "__main__":
    rng = np.random.default_rng(0)
    ins = {
        "hidden_states": rng.standard_normal((B, S, D), dtype=np.float32),
        "q_proj_w": rng.standard_normal((H * QHD, D), dtype=np.float32) * D ** -0.5,
        "kv_a_proj_w": rng.standard_normal((KV_RANK + ROPE, D), dtype=np.float32) * D ** -0.5,
        "kv_a_norm_w": np.ones(KV_RANK, dtype=np.float32),
        "kv_b_proj_w": rng.standard_normal((H * (NOPE + VD), KV_RANK), dtype=np.float32) * KV_RANK ** -0.5,
        "o_proj_w": rng.standard_normal((D, H * VD), dtype=np.float32) * (H * VD) ** -0.5,
    }
    out = kernel(**ins)
    print(out.shape, out.dtype, float(np.abs(out).mean()))


# revision 28
# speedup vs baseline: 332.4897x; 5.3377x over previous
"""Trainium2 Bass kernel for DeepSeek-style MLA (multi-head latent attention).

Sharding: 8 cores = 2 (batch) x 4 (head-groups of 4 heads).
Core c handles batch b = c // 4 and heads [4*(c%4), 4*(c%4)+4).
Each core computes its 4 heads' full attention + its partial o_proj
contribution y_partial [S, D] (bf16); host sums the 4 partials per batch.

v3 design (vs v2 baseline at ~295us):
  - Latent (ckv) projection sharded 4-way across the cores of each batch
    group: core c computes only ckv rows [128*(c%4), 128*(c%4)+128) (the
    chunk is baked into its wA input data; the program stays SPMD-pure).
    Per q-super, chunks are exchanged with an HBM AllGather over replica
    groups [[0..3],[4..7]] and read back in global chunk order, so no
    instruction depends on the core id.  This removes 4.6 GFLOP/core of
    redundant projection work (A phase: 11 m-tiles -> 8 per super).
  - RMS norm moved after the gather (sum-of-squares via ones-matmul over
    the 4 gathered chunks, rsqrt on ACT, scale on DVE) inside phase B.
  - Phase C starts with q-super 0 (needs only B(0)) and B(3) is emitted
    inside phase C behind C(qt=0), hiding the last gather's latency.
  - Everything else follows v2: bf16 matmuls with fp32 PSUM, partition
    broadcasts via all-ones lhsT matmuls, exact 128-granular causal
    narrowing, o_proj interleaved one q-super behind attention.
"""

import math
import sys

import numpy as np

for _p in ("/opt/trn_rl_repo",):
    if _p not in sys.path:
        sys.path.insert(0, _p)

# ---- problem constants (hardcoded per contract) ----
B = 2
S = 2048
D = 2048
H = 16
NOPE = 128
ROPE = 64
VD = 128
KV_RANK = 512
QHD = NOPE + ROPE
EPS = 1e-6
BASE = 10000.0

HPC = 4            # heads per core
NCORES = 8
P = 128
QS = 512           # q-super width
NQ = S // QS       # 4
NST = S // P       # 16 s-tiles
NKC = D // P       # 16 d-chunks
RC = KV_RANK // P  # 4 r-chunks
HALF = ROPE // 2   # 32

# wA column layout: 4x q-nope(128) | 2x q-pe pair(128) | kpe-w(64) | ckv-chunk(128)
WQ = HPC * QHD       # 768 (q cols)
WKPE = WQ            # 768, offset of kpe cols
WCKV = WQ + ROPE     # 832, offset of this core's ckv chunk cols
WCOLS = WCKV + P     # 960
# contiguous weight DMA blocks (col offset, width), in consumption order
WBLOCKS = [(0, 256), (256, 256), (512, 256), (768, 192)]

CC_GROUPS = [[0, 1, 2, 3], [4, 5, 6, 7]]

_CACHE = {}


def _declare_io(nc):
    import concourse.mybir as mybir

    f32 = mybir.dt.float32
    bf16 = mybir.dt.bfloat16
    io = {}
    io["hT"] = nc.dram_tensor("hT", [D, S], bf16, kind="ExternalInput").ap()
    io["wA"] = nc.dram_tensor("wA", [D * WCOLS], bf16, kind="ExternalInput").ap()
    io["kvbk"] = nc.dram_tensor("kvbk", [P, RC, HPC * NOPE], bf16, kind="ExternalInput").ap()
    io["kvbv"] = nc.dram_tensor("kvbv", [P, RC, HPC * VD], bf16, kind="ExternalInput").ap()
    io["owT"] = nc.dram_tensor("owT", [P, HPC, D], bf16, kind="ExternalInput").ap()
    io["cosT2"] = nc.dram_tensor("cosT2", [P, S], bf16, kind="ExternalInput").ap()
    io["sinT2"] = nc.dram_tensor("sinT2", [P, S], bf16, kind="ExternalInput").ap()
    io["masks"] = nc.dram_tensor("masks", [P, 4, QS], bf16, kind="ExternalInput").ap()
    io["y"] = nc.dram_tensor("y", [S, D], bf16, kind="ExternalOutput").ap()
    return io


def _declare_cc(nc, sfx=""):
    import concourse.mybir as mybir

    bf16 = mybir.dt.bfloat16
    part = nc.dram_tensor("ckv_part" + sfx, [NQ, P, QS], bf16, kind="Internal").ap()
    gath = nc.dram_tensor("ckv_gath" + sfx, [NQ, RC, P, QS], bf16, kind="Internal").ap()
    return part, gath


def _emit(nc, tc, io, sfx=""):
    """Emit the whole per-core program into TileContext tc."""
    import concourse.mybir as mybir

    f32 = mybir.dt.float32
    fr = mybir.dt.float32r
    bf16 = mybir.dt.bfloat16
    AF = mybir.ActivationFunctionType

    hT = io["hT"]; wA = io["wA"]
    kvbk = io["kvbk"]; kvbv = io["kvbv"]; owT = io["owT"]
    cosT2 = io["cosT2"]; sinT2 = io["sinT2"]; masks = io["masks"]
    y = io["y"]
    ccp, ccg = _declare_cc(nc, sfx)

    # ---- long-lived pools, strictly nested (LIFO exit) ----
    p_const_cm = tc.tile_pool(name="const" + sfx, bufs=1)
    p_const = p_const_cm.__enter__()
    p_qt_cm = tc.tile_pool(name="qt" + sfx, bufs=1)        # QTn/QTp/kpeT
    p_qt = p_qt_cm.__enter__()
    p_kvb_cm = tc.tile_pool(name="kvb" + sfx, bufs=1)      # KTn/Vsb: B -> attn
    p_kvb = p_kvb_cm.__enter__()
    p_bw_cm = tc.tile_pool(name="Bw" + sfx, bufs=1)        # kv_b weights
    p_bw = p_bw_cm.__enter__()
    p_ckv_cm = tc.tile_pool(name="ckv" + sfx, bufs=1)      # ckvT: gather -> B
    p_ckv = p_ckv_cm.__enter__()
    p_cs_cm = tc.tile_pool(name="cossin" + sfx, bufs=1)    # cos/sin: A only
    p_cs = p_cs_cm.__enter__()

    ones_pp_f = p_const.tile([P, P], f32, name="ones_pp")
    nc.vector.memset(ones_pp_f, 1.0)
    ones_pp = ones_pp_f.bitcast(fr)
    eps_sb = p_const.tile([P, 1], f32, name="eps")
    nc.vector.memset(eps_sb, EPS)
    mask_sb = p_const.tile([P, 4, QS], bf16, name="masks")

    QTn = p_qt.tile([P, HPC, S], bf16, name="QTn")
    QTp = p_qt.tile([P, 2, S], bf16, name="QTp")
    # kpeT duplicated on partitions [0:64] and [64:128] (lhsT base must
    # match rhs base per head parity)
    kpeT = p_qt.tile([P, S], bf16, name="kpeT")
    ckvT = p_ckv.tile([P, RC, S], bf16, name="ckvT")
    KTn = p_kvb.tile([P, HPC, S], bf16, name="KTn")
    Vsb = p_kvb.tile([P, NST, HPC, VD], bf16, name="Vsb")
    kvbk_sb = p_bw.tile([P, RC, HPC * NOPE], bf16, name="kvbk")
    kvbv_sb = p_bw.tile([P, RC, HPC * VD], bf16, name="kvbv")
    cos_sb = p_cs.tile([P, S], bf16, name="cos")
    sin_sb = p_cs.tile([P, S], bf16, name="sin")

    # ============ Phase A: projections, one super at a time =============
    # A groups (pairs of m-tiles, 2 banks each): [qn0,qn1] [qn2,qn3]
    # [pe0,pe1] [kpe,ckv_own].  PSUM: psA 2x2 + stats 1 + psB 3 = 8 banks.
    # After each super's A groups, the core's ckv chunk is pushed to HBM
    # and AllGather'ed; phase B for super sc is emitted after super sc+1's
    # A groups so the gather latency hides behind projection work.  B(3)
    # is emitted later, behind C(qt=0), with its own nested pools.
    p_A_cms = [tc.tile_pool(name="Ah" + sfx, bufs=6),
               tc.tile_pool(name="Aw" + sfx, bufs=1),
               tc.tile_pool(name="Ar" + sfx, bufs=2),
               tc.tile_pool(name="Ack" + sfx, bufs=2),
               tc.tile_pool(name="Bsq" + sfx, bufs=2),
               tc.tile_pool(name="Brs" + sfx, bufs=2),
               tc.tile_pool(name="psA" + sfx, bufs=2, space="PSUM"),
               tc.tile_pool(name="psS" + sfx, bufs=1, space="PSUM"),
               tc.tile_pool(name="psB" + sfx, bufs=3, space="PSUM")]
    p_hq, p_wa, p_rope, p_ckvo, p_sq, p_rs, psA, psS, psB = [
        cm.__enter__() for cm in p_A_cms]

    wa_sb = p_wa.tile([P, NKC, WCOLS], bf16, name="wa")

    def wa_dma(b, half=None, eng=None):
        eng = eng or nc.sync
        c0, cw = WBLOCKS[b]
        off = D * c0
        src_ = wA[off:off + D * cw].rearrange(
            "(kk p c) -> p kk c", p=P, c=cw)
        if half is None:
            eng.dma_start(out=wa_sb[:, :, c0:c0 + cw], in_=src_)
        else:
            k0, k1 = (0, NKC // 2) if half == 0 else (NKC // 2, NKC)
            eng.dma_start(out=wa_sb[:, k0:k1, c0:c0 + cw],
                          in_=src_[:, k0:k1, :])

    def hq_dma(sc, half, split=False, eng=None):
        eng = eng or nc.sync
        t = p_hq.tile([P, NKC // 2, QS], bf16, name="hq")
        src = hT[half * 1024:(half + 1) * 1024,
                 sc * QS:(sc + 1) * QS].rearrange("(kk p) s -> p kk s", p=P)
        if split:  # startup: land the first k-chunks earlier
            nc.sync.dma_start(out=t[:, 0:4, :], in_=src[:, 0:4, :])
            nc.sync.dma_start(out=t[:, 4:8, :], in_=src[:, 4:8, :])
        else:
            eng.dma_start(out=t, in_=src)
        return t

    # startup, all on SP in consumption order ([kpe,ckv] group first: wa
    # block 3, hq, cos/sin).  Keeping startup on SP lets the NEXT rep's
    # loads prefetch during this rep's attention phase (SP idles there),
    # so back-to-back reps pipeline.
    wa_dma(3)
    pend = [[hq_dma(0, 0, split=True)]]
    pend[0].append(hq_dma(0, 1))
    nc.sync.dma_start(out=cos_sb, in_=cosT2)
    nc.sync.dma_start(out=sin_sb, in_=sinT2)
    wa_dma(0, half=0)
    wa_dma(0, half=1)
    wa_dma(1)
    pend.append([hq_dma(1, 0)])
    wa_dma(2)
    pend[1].append(hq_dma(1, 1))
    pend.append([hq_dma(2, 0), hq_dma(2, 1)])
    # prefetch phase-B weights + masks during A
    for rc in range(RC):
        nc.sync.dma_start(out=kvbk_sb[:, rc, :], in_=kvbk[:, rc, :])
        nc.sync.dma_start(out=kvbv_sb[:, rc, :], in_=kvbv[:, rc, :])
    nc.sync.dma_start(out=mask_sb, in_=masks)

    # m-tile groups: (kind, idx); [kpe, ckv] first so the gather starts early
    GROUPS = [
        [("kpe", 0), ("ckv", 0)],
        [("qn", 0), ("qn", 1)],
        [("qn", 2), ("qn", 3)],
        [("pe", 0), ("pe", 1)],
    ]

    def mcol(kind, idx):
        if kind == "qn":
            return idx * P, P
        if kind == "pe":
            return (4 + idx) * P, P
        if kind == "kpe":
            return WKPE, ROPE
        return WCKV, P

    def rope_psum(src, part_hi, q0, dsts):
        """RoPE from PSUM src [part_hi, QS] -> each dst slice (bf16).
        cos/sin rows repeat every 64 partitions."""
        rot = p_rope.tile([P, QS], f32, name="rot")
        t1 = p_rope.tile([P, QS], f32, name="t1")
        for b0 in range(0, part_hi, ROPE):
            nc.vector.tensor_scalar_mul(
                out=rot[b0:b0 + HALF], in0=src[b0 + HALF:b0 + ROPE],
                scalar1=-1.0)
            nc.vector.tensor_copy(
                out=rot[b0 + HALF:b0 + ROPE], in_=src[b0:b0 + HALF])
        csl = cos_sb[:part_hi, q0:q0 + QS]
        ssl = sin_sb[:part_hi, q0:q0 + QS]
        nc.vector.tensor_mul(t1[:part_hi], src, csl)
        nc.vector.tensor_mul(rot[:part_hi], rot[:part_hi], ssl)
        for dst in dsts:
            nc.vector.tensor_add(dst, t1[:part_hi], rot[:part_hi])

    def emit_A_group(sc, hq, group):
        q0 = sc * QS
        ps = psA.tile([P, 2, QS], f32, name="psA")
        for k in range(NKC):
            rhs = hq[k // 8][:, k % 8, :]
            for i, (kind, idx) in enumerate(group):
                c0, cw = mcol(kind, idx)
                nc.tensor.matmul(
                    ps[:cw, i, :], wa_sb[:, k, c0:c0 + cw], rhs,
                    start=(k == 0), stop=(k == NKC - 1))
        for i, (kind, idx) in enumerate(group):
            if kind == "qn":
                nc.scalar.copy(out=QTn[:, idx, q0:q0 + QS],
                               in_=ps[:, i, :])
            elif kind == "pe":
                rope_psum(ps[:, i, :], P, q0,
                          [QTp[:, idx, q0:q0 + QS]])
            elif kind == "kpe":
                rope_psum(ps[:ROPE, i, :], ROPE, q0,
                          [kpeT[:ROPE, q0:q0 + QS],
                           kpeT[ROPE:, q0:q0 + QS]])
            else:  # this core's ckv chunk -> HBM -> AllGather
                own = p_ckvo.tile([P, QS], bf16, name="ckv_own")
                nc.scalar.copy(out=own, in_=ps[:, i, :])
                # gpsimd queue: right before its collective, keeping
                # descriptor generation off the SP/ACT queues.
                nc.gpsimd.dma_start(out=ccp[sc], in_=own)
                nc.gpsimd.collective_compute(
                    "AllGather", mybir.AluOpType.bypass,
                    replica_groups=CC_GROUPS, ins=[ccp[sc]],
                    outs=[ccg[sc]])
                for rc in range(RC):
                    # SP queue: idle after startup, and keeping these
                    # off gpsimd stops their descriptor generation from
                    # delaying the next super's gather.
                    nc.sync.dma_start(out=ckvT[:, rc, q0:q0 + QS],
                                      in_=ccg[sc, rc])

    def emit_B_super(sc, pools, tag="psb"):
        b_sq, b_rs, b_psS, b_psB = pools
        q0 = sc * QS
        # RMS over the gathered full latent: sumsq via ones-matmul,
        # rsqrt broadcast already landed across partitions by the matmul.
        ps_ss = b_psS.tile([P, QS], f32, name=tag)
        for rc in range(RC):
            sq = b_sq.tile([P, QS], fr, name="sq")
            nc.scalar.activation(out=sq, in_=ckvT[:, rc, q0:q0 + QS],
                                 func=AF.Square)
            nc.tensor.matmul(ps_ss, ones_pp, sq,
                             start=(rc == 0), stop=(rc == RC - 1))
        s_b = b_rs.tile([P, QS], f32, name="s_b")
        nc.scalar.activation(out=s_b, in_=ps_ss, func=AF.Sqrt,
                             bias=eps_sb, scale=1.0 / KV_RANK)
        rs_b = b_rs.tile([P, QS], f32, name="rs_b")
        nc.vector.reciprocal_approx_fast(out=rs_b, in_=s_b)
        for rc in range(RC):
            nc.vector.tensor_mul(ckvT[:, rc, q0:q0 + QS],
                                 ckvT[:, rc, q0:q0 + QS], rs_b)
        for h in range(HPC):
            ps = b_psB.tile([P, QS], f32, name=tag)
            for rc in range(RC):
                nc.tensor.matmul(
                    ps, kvbk_sb[:, rc, h * NOPE:(h + 1) * NOPE],
                    ckvT[:, rc, sc * QS:(sc + 1) * QS],
                    start=(rc == 0), stop=(rc == RC - 1))
            nc.scalar.copy(out=KTn[:, h, sc * QS:(sc + 1) * QS], in_=ps)
        for stl in range(4):
            st = sc * 4 + stl
            psv = b_psB.tile([P, QS], f32, name=tag)
            for rc in range(RC):
                nc.tensor.matmul(
                    psv, ckvT[:, rc, st * P:(st + 1) * P],
                    kvbv_sb[:, rc, :],
                    start=(rc == 0), stop=(rc == RC - 1))
            nc.scalar.copy(out=Vsb[:, st, :, :],
                           in_=psv.rearrange("p (h v) -> p h v", h=HPC))

    # ckv groups lead the q groups by ~2 supers, so each AllGather has
    # ~2 supers (~55us) of projection work to hide behind before its
    # B phase consumes the gathered chunks.
    hq = {sc: pend[sc] for sc in range(3)}
    bpools = (p_sq, p_rs, psS, psB)
    emit_A_group(0, hq[0], GROUPS[0])
    emit_A_group(1, hq[1], GROUPS[0])
    hq[3] = [hq_dma(3, 0), hq_dma(3, 1)]
    for g in (1, 2, 3):
        emit_A_group(0, hq[0], GROUPS[g])
    emit_A_group(2, hq[2], GROUPS[0])
    for g in (1, 2, 3):
        emit_A_group(1, hq[1], GROUPS[g])
    emit_B_super(0, bpools)
    emit_A_group(3, hq[3], GROUPS[0])
    for g in (1, 2, 3):
        emit_A_group(2, hq[2], GROUPS[g])
    emit_B_super(1, bpools)
    for g in (1, 2, 3):
        emit_A_group(3, hq[3], GROUPS[g])

    for cm in reversed(p_A_cms):
        cm.__exit__(None, None, None)
    p_cs_cm.__exit__(None, None, None)  # free cos/sin

    p_ow_cm = tc.tile_pool(name="ow" + sfx, bufs=1)
    p_ow = p_ow_cm.__enter__()
    owT_sb = p_ow.tile([P, HPC, D], bf16, name="owT")
    p_at_cm = tc.tile_pool(name="at" + sfx, bufs=1)
    p_at = p_at_cm.__enter__()
    at = p_at.tile([P, HPC, S], bf16, name="at")
    for hc in range(HPC):
        nc.sync.dma_start(out=owT_sb[:, hc, :], in_=owT[:, hc, :])

    # ============ Phase C: attention + o_proj, per q-super ================
    # B(3) is emitted behind C(qt=0) so the last gather hides under
    # attention.  PSUM: psSC 2 + psPV 3 + psM 1 + (psB 2 until B(3) done,
    # then psD 2) = 8 banks.
    with tc.tile_pool(name="Ce" + sfx, bufs=4) as pEP, \
         tc.tile_pool(name="Ca" + sfx, bufs=2) as pEacc, \
         tc.tile_pool(name="Cr" + sfx, bufs=2) as pRec, \
         tc.tile_pool(name="psC" + sfx, bufs=2, space="PSUM") as psSC, \
         tc.tile_pool(name="psP" + sfx, bufs=3, space="PSUM") as psPV, \
         tc.tile_pool(name="psM" + sfx, bufs=1, space="PSUM") as psM:

        def emit_attn_super(qt, post=None):
            q0 = qt * QS
            nj = 4 * qt + 4
            for h in range(HPC):
                hp = (h % 2) * ROPE
                qprhs = QTp[hp:hp + ROPE, h // 2, :]
                E_acc = pEacc.tile([P, QS], fr, name="E_acc")
                ps_pv = psPV.tile([P, QS], f32, name="ps_pv")
                for j in range(nj):
                    jl = j - 4 * qt
                    off = max(jl, 0) * P
                    ps_sc = psSC.tile([P, QS], f32, name="ps_sc")
                    nc.tensor.matmul(
                        ps_sc[:, off:], KTn[:, h, j * P:(j + 1) * P],
                        QTn[:, h, q0 + off:q0 + QS], start=True, stop=False)
                    nc.tensor.matmul(
                        ps_sc[:, off:],
                        kpeT[hp:hp + ROPE, j * P:(j + 1) * P],
                        qprhs[:, q0 + off:q0 + QS], start=False, stop=True)
                    ep = pEP.tile([P, QS], bf16, name="ep")
                    nc.scalar.activation(out=ep[:, off:], in_=ps_sc[:, off:],
                                         func=AF.Exp)
                    if jl >= 0:  # diagonal super-block: causal mask
                        nc.vector.tensor_mul(ep[:, off:], ep[:, off:],
                                             mask_sb[:, jl, off:])
                    if j == 0:
                        nc.vector.tensor_copy(out=E_acc, in_=ep)
                    else:
                        nc.vector.tensor_add(E_acc[:, off:], E_acc[:, off:],
                                             ep[:, off:])
                    nc.tensor.matmul(ps_pv[:, off:], Vsb[:, j, h, :],
                                     ep[:, off:], start=(j == 0),
                                     stop=(jl == 3))
                # broadcast column sums on PE, 1/x on DVE, fused drain
                ps_sums = psM.tile([P, QS], f32, name="ps_sums")
                nc.tensor.matmul(ps_sums, ones_pp, E_acc, start=True, stop=True)
                rec = pRec.tile([P, QS], f32, name="rec")
                nc.vector.reciprocal_approx_fast(out=rec, in_=ps_sums)
                nc.vector.tensor_mul(at[:, h, q0:q0 + QS], ps_pv, rec)
                if post is not None:
                    post(h)

        # C(qt=0) first (needs only B(0)); B(3) is emitted after C(qt=1)
        # sharing psD's bank pair (same tile tag), so the last gather hides
        # under ~35us of attention work.
        emit_attn_super(0)
        with tc.tile_pool(name="B3sq" + sfx, bufs=2) as b_sq, \
             tc.tile_pool(name="B3rs" + sfx, bufs=2) as b_rs, \
             tc.tile_pool(name="Dy" + sfx, bufs=4) as pDy, \
             tc.tile_pool(name="psD" + sfx, bufs=2, space="PSUM") as psD:

            def emit_oproj_st(qt, stl):
                st = qt * 4 + stl
                for nk in range(NQ):
                    psy = psD.tile([P, QS], f32, name="psy")
                    for hc in range(HPC):
                        nc.tensor.matmul(
                            psy, at[:, hc, st * P:(st + 1) * P],
                            owT_sb[:, hc, nk * QS:(nk + 1) * QS],
                            start=(hc == 0), stop=(hc == HPC - 1))
                    ys = pDy.tile([P, QS], bf16, name="ys")
                    nc.scalar.copy(out=ys, in_=psy)
                    nc.gpsimd.dma_start(
                        out=y[st * P:(st + 1) * P, nk * QS:(nk + 1) * QS],
                        in_=ys)

            emit_B_super(2, (b_sq, b_rs, psD, psD), tag="psy")
            for qt in range(1, NQ):
                emit_attn_super(qt, post=lambda h, q=qt: emit_oproj_st(q - 1, h))
                if qt == 1:
                    emit_B_super(NQ - 1, (b_sq, b_rs, psD, psD), tag="psy")
            for stl in range(4):
                emit_oproj_st(NQ - 1, stl)

    p_at_cm.__exit__(None, None, None)
    p_ow_cm.__exit__(None, None, None)
    p_ckv_cm.__exit__(None, None, None)
    p_bw_cm.__exit__(None, None, None)
    p_kvb_cm.__exit__(None, None, None)
    p_qt_cm.__exit__(None, None, None)
    p_const_cm.__exit__(None, None, None)


def _build_program(reps=1):
    import concourse.bacc as bacc
    import concourse.tile as tile

    nc = bacc.Bacc("TRN2", target_bir_lowering=False, debug=False,
                   num_devices=NCORES)
    with tile.TileContext(nc) as tc:
        io = _declare_io(nc)
        for r in range(reps):
            _emit(nc, tc, io, sfx=f"_r{r}" if reps > 1 else "")
    nc.compile()
    return nc


def _rope_cos_sin():
    inv_freq = 1.0 / (BASE ** (np.arange(0, ROPE, 2, dtype=np.float32) / ROPE))
    t = np.arange(S, dtype=np.float32)
    freqs = np.outer(t, inv_freq)                     # [S, ROPE/2]
    emb = np.concatenate([freqs, freqs], axis=-1)     # [S, ROPE]
    return np.cos(emb), np.sin(emb)


def _host_prep(hidden_states, q_proj_w, kv_a_proj_w, kv_a_norm_w,
               kv_b_proj_w, o_proj_w):
    """Build per-core input maps (bf16 operands)."""
    import ml_dtypes

    bf = ml_dtypes.bfloat16
    hidden_states = np.asarray(hidden_states, dtype=np.float32)
    q_proj_w = np.asarray(q_proj_w, dtype=np.float32)
    kv_a_proj_w = np.asarray(kv_a_proj_w, dtype=np.float32)
    kv_a_norm_w = np.asarray(kv_a_norm_w, dtype=np.float32)
    kv_b_proj_w = np.asarray(kv_b_proj_w, dtype=np.float32)
    o_proj_w = np.asarray(o_proj_w, dtype=np.float32)

    scale = np.float32(1.0 / math.sqrt(QHD))
    qws = (q_proj_w * scale).reshape(H, QHD, D)
    kvb = (kv_b_proj_w * kv_a_norm_w[None, :]).reshape(H, NOPE + VD, KV_RANK)

    cos, sin = _rope_cos_sin()                             # [S, ROPE]
    cosT2 = np.ascontiguousarray(np.tile(cos.T, (2, 1))).astype(bf)  # [128, S]
    sinT2 = np.ascontiguousarray(np.tile(sin.T, (2, 1))).astype(bf)

    # diag masks, stored partition-major: masks[p, j, q]
    r = np.arange(P)[:, None]
    ql = np.arange(QS)[None, :]
    masks = np.stack([(ql >= j * P + r).astype(np.float32) for j in range(4)])
    masks = np.ascontiguousarray(masks.transpose(1, 0, 2)).astype(bf)

    in_maps = []
    for c in range(NCORES):
        b, g = divmod(c, HPC)
        heads = list(range(HPC * g, HPC * g + HPC))
        hT = np.ascontiguousarray(hidden_states[b].T).astype(bf)   # [D, S]
        # wA cols: 4x nope(128), 2x pe-pair(128), kpe-w(64), ckv-chunk(128)
        cols = np.concatenate(
            [qws[h, :NOPE, :] for h in heads]
            + [qws[h, NOPE:, :] for h in heads]
            + [kv_a_proj_w[KV_RANK:, :],
               kv_a_proj_w[g * P:(g + 1) * P, :]],
            axis=0)                                        # [960, D]
        wA_full = cols.T                                   # [D, 960]
        # flat, block-contiguous in WBLOCKS order (device DMAs per block)
        wA_c = np.concatenate(
            [np.ascontiguousarray(wA_full[:, c0:c0 + cw]).ravel()
             for c0, cw in WBLOCKS]).astype(bf)
        # kvbk [128, 4, 4*128]: kvbk[p, rc, h*128+j] = kvb[heads[h], j, rc*128+p]
        kn = np.stack([kvb[h, :NOPE, :] for h in heads])    # [h, j, r]
        kvbk_c = np.ascontiguousarray(
            kn.transpose(2, 0, 1).reshape(RC, P, HPC, NOPE)
            .transpose(1, 0, 2, 3).reshape(P, RC, HPC * NOPE)).astype(bf)
        kv = np.stack([kvb[h, NOPE:, :] for h in heads])    # [h, j(vd), r]
        kvbv_c = np.ascontiguousarray(
            kv.transpose(2, 0, 1).reshape(RC, P, HPC, VD)
            .transpose(1, 0, 2, 3).reshape(P, RC, HPC * VD)).astype(bf)
        # owT [128, 4, D]: owT[p, hc, n] = o_proj_w[n, g*512 + hc*128 + p]
        ow = o_proj_w[:, g * HPC * VD:(g + 1) * HPC * VD]   # [D, 512]
        owT_c = np.ascontiguousarray(
            ow.T.reshape(HPC, VD, D).transpose(1, 0, 2)).astype(bf)
        in_maps.append({
            "hT": hT, "wA": wA_c,
            "kvbk": kvbk_c, "kvbv": kvbv_c, "owT": owT_c,
            "cosT2": cosT2, "sinT2": sinT2, "masks": masks,
        })
    return in_maps


def _gather(results):
    out = np.zeros((B, S, D), dtype=np.float32)
    for c in range(NCORES):
        out[c // HPC] += np.asarray(results[c]["y"], dtype=np.float32)
    return out


def kernel(hidden_states, q_proj_w, kv_a_proj_w, kv_a_norm_w,
           kv_b_proj_w, o_proj_w):
    from concourse import bass_utils

    in_maps = _host_prep(hidden_states, q_proj_w, kv_a_proj_w, kv_a_norm_w,
                         kv_b_proj_w, o_proj_w)
    if "nc" not in _CACHE:
        _CACHE["nc"] = _build_program()
    nc = _CACHE["nc"]
    res = bass_utils.run_bass_kernel_spmd(nc, in_maps, list(range(NCORES)))
    return _gather(res.results)


if __name__ == "__main__":
    rng = np.random.default_rng(0)
    ins = {
        "hidden_states": rng.standard_normal((B, S, D), dtype=np.float32),
        "q_proj_w": rng.standard_normal((H * QHD, D), dtype=np.float32) * D ** -0.5,
        "kv_a_proj_w": rng.standard_normal((KV_RANK + ROPE, D), dtype=np.float32) * D ** -0.5,
        "kv_a_norm_w": np.ones(KV_RANK, dtype=np.float32),
        "kv_b_proj_w": rng.standard_normal((H * (NOPE + VD), KV_RANK), dtype=np.float32) * KV_RANK ** -0.5,
        "o_proj_w": rng.standard_normal((D, H * VD), dtype=np.float32) * (H * VD) ** -0.5,
    }
    out = kernel(**ins)
    print(out.shape, out.dtype, float(np.abs(out).mean()))
